# revision 1
# baseline (speedup 1.0000x reference)
"""Fused attention kernel for trn2, 8 NeuronCores.

Problem: nn_Attention (b=2, n=2048, dim=1024, heads=16, dim_head=64).
  y = ((softmax(mask(qn @ kn^T)) @ v) @ Wo)   with LN(x) input, qk-RMS-norm.

Sharding: 8 cores = 2 batches x 4 head-groups (4 heads each).
Data parallel over batch; tensor parallel over heads (Wq/Wkv column-slice,
Wo row-slice; partial outputs summed on host).

Per-core pipeline (all matmuls float32r = TF32-ish, fp32 accumulate):
  LN (bn_stats -> exp(-0.5 ln(var+eps))) -> PE-transpose xn -> xnT
  qT/kT proj ([e,n] layout), v proj ([n,e] layout)
  q/k RMS norm: sumsq via block-ones matmul, rstd = 1/sqrt on ACT+DVE,
    row-broadcast via DRAM round-trip, fold 8*gamma
  scores s^T[j,i] per head (2 heads row-concurrent on PE, separate PSUM
    banks), exp on ACT with mask bias fused (no max subtraction needed:
    |q^|=|k^|=8 bounds |s|<=64 so exp cannot overflow)
  attn@v with a ones column appended to v (M=65) so row 64 of each PSUM
    accumulator is the softmax denominator for free; 1/denom via
    reciprocal_approx_fast after a DRAM row-broadcast; out-proj
Output: per-core outT [1024, 2048]; host sums groups and transposes.
"""
import numpy as np

import concourse.bass as bass
import concourse.mybir as mybir
import concourse.tile as tile
from concourse import bacc
from concourse.bass_utils import run_bass_kernel_spmd
from concourse.masks import make_identity

dt = mybir.dt
AF = mybir.ActivationFunctionType
ALU = mybir.AluOpType

B, N, DIM = 2, 2048, 1024
HEADS, D = 16, 64
G = 4            # head groups (cores per batch)
HPG = 4          # heads per group
E = HPG * D      # 256 cols per group
NT = N // 128    # 16 row tiles
CT = DIM // 128  # 8 contraction tiles
NEG = -1.0e30

_CACHE: dict = {}
STOP_AFTER = None  # "ln", "proj", "v", "attn"


def _build_nc():
    nc = bacc.Bacc()
    x_d = nc.dram_tensor("x", [N, DIM], dt.float32, kind="ExternalInput")
    mb_d = nc.dram_tensor("mb", [128, NT], dt.float32, kind="ExternalInput")
    wq_d = nc.dram_tensor("wq", [DIM, E], dt.float32, kind="ExternalInput")
    wk_d = nc.dram_tensor("wk", [DIM, E], dt.float32, kind="ExternalInput")
    wv_d = nc.dram_tensor("wv", [DIM, E], dt.float32, kind="ExternalInput")
    wo_d = nc.dram_tensor("wo", [E, DIM], dt.float32, kind="ExternalInput")
    gq_d = nc.dram_tensor("gq", [128, 2], dt.float32, kind="ExternalInput")
    gk_d = nc.dram_tensor("gk", [128, 2], dt.float32, kind="ExternalInput")
    out_d = nc.dram_tensor("outT", [DIM, N], dt.float32, kind="ExternalOutput")

    with tile.TileContext(nc, pool_alloc_mode="queue") as tc:
        _emit(nc, tc, x_d, mb_d, wq_d, wk_d, wv_d, wo_d, gq_d, gk_d, out_d)
    nc.compile()
    return nc


def _emit(nc, tc, x_d, mb_d, wq_d, wk_d, wv_d, wo_d, gq_d, gk_d, out_d):
    from contextlib import ExitStack

    ctx = ExitStack()
    with ctx:
        const = ctx.enter_context(tc.tile_pool(name="const", bufs=1))
        drams = ctx.enter_context(tc.tile_pool(name="drams", bufs=1, space="DRAM"))

        # ---- constants ----
        ident_f = const.tile([128, 128], dt.float32, tag="identf")
        make_identity(nc, ident_f)
        ident = const.tile([128, 128], dt.float32r, tag="ident")
        nc.vector.tensor_copy(ident, ident_f)

        ones_f = const.tile([128, 4], dt.float32, tag="onesf")
        nc.vector.memset(ones_f, 1.0)
        ones4 = const.tile([128, 4], dt.float32r, tag="ones4")
        nc.vector.tensor_copy(ones4, ones_f)

        # blockones[et] [128, 4]: col (2*et+h) has ones in rows h*64:(h+1)*64
        blk = []
        for et in range(2):
            bf = const.tile([128, 4], dt.float32, tag=f"blkf{et}", name=f"blkf{et}")
            nc.vector.memset(bf, 0.0)
            nc.vector.memset(bf[0:64, 2 * et : 2 * et + 1], 1.0)
            nc.vector.memset(bf[64:128, 2 * et + 1 : 2 * et + 2], 1.0)
            br = const.tile([128, 4], dt.float32r, tag=f"blk{et}", name=f"blk{et}")
            nc.vector.tensor_copy(br, bf)
            blk.append(br)

        eps_sb = const.tile([128, 1], dt.float32, tag="eps")
        nc.vector.memset(eps_sb, 1e-5)

        mb_sb = const.tile([128, NT], dt.float32, tag="mb")
        nc.sync.dma_start(out=mb_sb, in_=mb_d[:, :])
        gq_sb = const.tile([128, 2], dt.float32, tag="gq")
        nc.sync.dma_start(out=gq_sb, in_=gq_d[:, :])
        gk_sb = const.tile([128, 2], dt.float32, tag="gk")
        nc.sync.dma_start(out=gk_sb, in_=gk_d[:, :])
        igq_sb = const.tile([128, 2], dt.float32, tag="igq")
        nc.vector.reciprocal(igq_sb, gq_sb)
        igk_sb = const.tile([128, 2], dt.float32, tag="igk")
        nc.vector.reciprocal(igk_sb, gk_sb)

        # ---- persistent activations ----
        vqk = ctx.enter_context(tc.tile_pool(name="vqk", bufs=1))
        v_sb = [vqk.tile([128, HPG, 65], dt.float32r, tag=f"v{nt}", name=f"v{nt}") for nt in range(NT)]
        # qn/kn double as the pre-norm projection buffers (normed in place)
        qn = [vqk.tile([128, N], dt.float32r, tag=f"qn{et}", name=f"qn{et}") for et in range(2)]
        kn = [vqk.tile([128, N], dt.float32r, tag=f"kn{et}", name=f"kn{et}") for et in range(2)]

        rstd_dram = drams.tile([8, N], dt.float32)

        # ---- weights (fp32 -> f32r cast during SWDGE DMA) ----
        wqkv_cm = tc.tile_pool(name="wqkv", bufs=1)
        wqkv = wqkv_cm.__enter__()
        wq_sb = wqkv.tile([128, CT, E], dt.float32r, tag="wq")
        nc.gpsimd.dma_start(out=wq_sb, in_=wq_d.rearrange("(ct p) e -> p ct e", p=128))
        wk_sb = wqkv.tile([128, CT, E], dt.float32r, tag="wk")
        nc.gpsimd.dma_start(out=wk_sb, in_=wk_d.rearrange("(ct p) e -> p ct e", p=128))
        wv_sb = wqkv.tile([128, CT, E], dt.float32r, tag="wv")
        nc.gpsimd.dma_start(out=wv_sb, in_=wv_d.rearrange("(ct p) e -> p ct e", p=128))

        xnT_cm = tc.tile_pool(name="xnT", bufs=1)
        xnT_pool = xnT_cm.__enter__()
        xnT = [xnT_pool.tile([128, N], dt.float32r, tag=f"xnT{ct}", name=f"xnT{ct}") for ct in range(CT)]

        # ================= Phase 1: LN + transpose =================
        with tc.tile_pool(name="xp", bufs=4) as xp, \
             tc.tile_pool(name="xnp", bufs=6) as xnp, \
             tc.tile_pool(name="stp", bufs=6) as stp, \
             tc.tile_pool(name="pstr", bufs=1, space="PSUM") as pstr:
            for ntg in range(NT // 4):
                xns = []
                for k in range(4):
                    nt = ntg * 4 + k
                    xt = xp.tile([128, DIM], dt.float32, tag="xt")
                    nc.sync.dma_start(out=xt, in_=x_d[nt * 128 : (nt + 1) * 128, :])
                    st = stp.tile([128, 2, 6], dt.float32, tag="st")
                    nc.vector.bn_stats(out=st[:, 0, :], in_=xt[:, 0:512])
                    nc.vector.bn_stats(out=st[:, 1, :], in_=xt[:, 512:1024])
                    mv = stp.tile([128, 2], dt.float32, tag="mv")
                    nc.vector.bn_aggr(out=mv, in_=st)
                    lnv = stp.tile([128, 1], dt.float32, tag="lnv")
                    nc.scalar.activation(lnv, mv[:, 1:2], AF.Sqrt, bias=eps_sb)
                    rstd = stp.tile([128, 1], dt.float32, tag="rstd")
                    nc.vector.reciprocal(rstd, lnv)
                    xnt = xnp.tile([128, DIM], dt.float32r, tag="xn")
                    nc.vector.tensor_scalar(
                        out=xnt, in0=xt, scalar1=mv[:, 0:1], scalar2=rstd,
                        op0=ALU.subtract, op1=ALU.mult,
                    )
                    xns.append(xnt)
                for ct in range(CT):
                    bank = pstr.tile([128, 512], dt.float32r, tag=f"tr{ct}", name=f"tr{ct}")
                    for k in range(4):
                        nc.tensor.transpose(
                            out=bank[:, k * 128 : (k + 1) * 128],
                            in_=xns[k][:, ct * 128 : (ct + 1) * 128],
                            identity=ident,
                        )
                    nc.scalar.copy(
                        out=xnT[ct][:, ntg * 512 : (ntg + 1) * 512], in_=bank
                    )

        if STOP_AFTER == "ln":
            xnT_cm.__exit__(None, None, None)
            wqkv_cm.__exit__(None, None, None)
            return
        # ================= Phase 2: q/k projections + qk-norm =================
        with tc.tile_pool(name="psqp", bufs=4, space="PSUM") as psqp, \
             tc.tile_pool(name="ssp", bufs=1, space="PSUM") as ssp, \
             tc.tile_pool(name="q2p", bufs=3) as q2p, \
             tc.tile_pool(name="rstp", bufs=1) as rstp, \
             tc.tile_pool(name="bcp", bufs=4) as bcp:
            rstdq_sb = rstp.tile([4, N], dt.float32, tag="rstdq")
            rstdk_sb = rstp.tile([4, N], dt.float32, tag="rstdk")

            for ti, (w_sb, g_sb, ig_sb, ts_tiles, rstd_sb) in enumerate((
                (wq_sb, gq_sb, igq_sb, qn, rstdq_sb),
                (wk_sb, gk_sb, igk_sb, kn, rstdk_sb),
            )):
                ssbs = [ssp.tile([4, 512], dt.float32, tag=f"ss{i}", name=f"ss{i}")
                        for i in range(4)]
                for et in range(2):
                    for ncn in range(4):
                        ps = psqp.tile([128, 512], dt.float32, tag="psq")
                        for ct in range(CT):
                            nc.tensor.matmul(
                                ps,
                                w_sb[:, ct, et * 128 : (et + 1) * 128],
                                xnT[ct][:, ncn * 512 : (ncn + 1) * 512],
                                start=(ct == 0), stop=(ct == CT - 1),
                            )
                        nc.vector.tensor_scalar_mul(
                            out=ts_tiles[et][:, ncn * 512 : (ncn + 1) * 512],
                            in0=ps, scalar1=g_sb[:, et : et + 1],
                        )
                        q2c = q2p.tile([128, 512], dt.float32r, tag="q2c")
                        nc.vector.scalar_tensor_tensor(
                            out=q2c, in0=ps, scalar=ig_sb[:, et : et + 1],
                            in1=ts_tiles[et][:, ncn * 512 : (ncn + 1) * 512],
                            op0=ALU.mult, op1=ALU.mult)
                        nc.tensor.matmul(
                            ssbs[ncn], blk[et], q2c,
                            start=(et == 0), stop=(et == 1),
                            skip_group_check=True,
                        )
                for ncn in range(4):
                    lnt = rstp.tile([4, 512], dt.float32, tag="lnt")
                    nc.scalar.activation(lnt, ssbs[ncn], AF.Sqrt)
                    nc.vector.reciprocal_approx_fast(
                        out=rstd_sb[:, ncn * 512 : (ncn + 1) * 512], in_=lnt)
                    nc.sync.dma_start(
                        out=rstd_dram[4 * ti : 4 * ti + 4,
                                      ncn * 512 : (ncn + 1) * 512],
                        in_=rstd_sb[:, ncn * 512 : (ncn + 1) * 512])
                # row-broadcast back via DRAM, chunked, in-place apply
                for et in range(2):
                    for ncn in range(4):
                        rb = bcp.tile([128, 512], dt.float32, tag="rb")
                        nc.sync.dma_start(
                            out=rb,
                            in_=bass.AP(
                                tensor=rstd_dram.tensor,
                                offset=rstd_dram.offset + (4 * ti + 2 * et) * N
                                + ncn * 512,
                                ap=[[N, 2], [0, 64], [1, 512]],
                            ),
                        )
                        sl = ts_tiles[et][:, ncn * 512 : (ncn + 1) * 512]
                        nc.vector.tensor_mul(out=sl, in0=sl, in1=rb)

        if STOP_AFTER == "proj":
            xnT_cm.__exit__(None, None, None)
            wqkv_cm.__exit__(None, None, None)
            return
        # ---- v projection (with ones column per head for denominators) ----
        with tc.tile_pool(name="psvp", bufs=3, space="PSUM") as psvp:
            for nt in range(NT):
                psv = psvp.tile([128, E], dt.float32, tag="psv")
                for ct in range(CT):
                    nc.tensor.matmul(
                        psv,
                        xnT[ct][:, nt * 128 : (nt + 1) * 128],
                        wv_sb[:, ct, :],
                        start=(ct == 0), stop=(ct == CT - 1),
                    )
                nc.vector.tensor_copy(
                    out=v_sb[nt][:, :, 0:64],
                    in_=psv.rearrange("p (h d) -> p h d", d=64),
                )
                nc.vector.tensor_copy(out=v_sb[nt][:, :, 64], in_=ones4)

        xnT_cm.__exit__(None, None, None)
        wqkv_cm.__exit__(None, None, None)

        if STOP_AFTER == "v":
            return
        # ================= Phase 3: attention =================
        outn_pool = ctx.enter_context(tc.tile_pool(name="outnp", bufs=1))
        outn = [outn_pool.tile([128, N], dt.float32r, tag=f"outn{et}", name=f"outn{et}")
                for et in range(2)]
        with tc.tile_pool(name="scp", bufs=2, space="PSUM") as scp, \
             tc.tile_pool(name="avp", bufs=2, space="PSUM") as avp, \
             tc.tile_pool(name="expp", bufs=3) as expp, \
             tc.tile_pool(name="dbp", bufs=3) as dbp:
            for et in range(2):
                for blki in range(4):
                    ofs = blki * 512
                    # av[h]: [65, 512] bank; rows 0-63 numer, row 64 denom
                    avA = avp.tile([65, 512], dt.float32, tag="avA")
                    avB = avp.tile([65, 512], dt.float32, tag="avB")
                    for jt in range(NT):
                        j0 = jt * 128
                        sc = scp.tile([128, 1024], dt.float32, tag="sc")
                        nc.tensor.matmul(
                            sc[:, 0:512], kn[et][0:64, j0 : j0 + 128],
                            qn[et][0:64, ofs : ofs + 512],
                            start=True, stop=True)
                        nc.tensor.matmul(
                            sc[:, 512:1024], kn[et][64:128, j0 : j0 + 128],
                            qn[et][64:128, ofs : ofs + 512],
                            start=True, stop=True)
                        ex = expp.tile([128, 1024], dt.float32r, tag="ex")
                        nc.scalar.activation(
                            ex, sc, AF.Exp, bias=mb_sb[:, jt : jt + 1])
                        st_, sp_ = (jt == 0), (jt == NT - 1)
                        nc.tensor.matmul(
                            avA[0:65, :], v_sb[jt][:, 2 * et, :], ex[:, 0:512],
                            start=st_, stop=sp_, skip_group_check=True)
                        nc.tensor.matmul(
                            avB[0:65, :], v_sb[jt][:, 2 * et + 1, :],
                            ex[:, 512:1024],
                            start=st_, stop=sp_, skip_group_check=True)
                    # denominators (row 64 of each bank) -> DRAM -> bcast
                    dn = dbp.tile([128, 1024], dt.float32, tag="dn")
                    nc.vector.tensor_copy(out=dn[64:65, 0:512], in_=avA[64:65, :])
                    nc.vector.tensor_copy(out=dn[64:65, 512:1024], in_=avB[64:65, :])
                    ddr = drams.tile([2, 512], dt.float32, tag=f"ddr{et}{blki}",
                                     name=f"ddr{et}{blki}")
                    nc.sync.dma_start(
                        out=ddr.rearrange("a b -> (a b)")[None, :],
                        in_=dn[64:65, :])
                    for h, avh in ((0, avA), (1, avB)):
                        db = dbp.tile([64, 512], dt.float32, tag=f"db{h}",
                                      name=f"db{h}")
                        nc.sync.dma_start(
                            out=db,
                            in_=bass.AP(
                                tensor=ddr.tensor,
                                offset=ddr.offset + h * 512,
                                ap=[[0, 64], [1, 512]],
                            ),
                        )
                        nc.vector.reciprocal_approx_fast(out=db, in_=db)
                        nc.vector.tensor_mul(
                            out=outn[et][h * 64 : (h + 1) * 64, ofs : ofs + 512],
                            in0=avh[0:64, :], in1=db)

        if STOP_AFTER == "attn":
            return
        # ================= Phase 4: output projection =================
        with tc.tile_pool(name="wop", bufs=1) as wop, \
             tc.tile_pool(name="pop", bufs=3, space="PSUM") as pop, \
             tc.tile_pool(name="otp", bufs=3) as otp:
            wo_sb = wop.tile([128, 2, DIM], dt.float32r, tag="wo")
            nc.gpsimd.dma_start(
                out=wo_sb, in_=wo_d.rearrange("(et p) c -> p et c", p=128))
            for ct in range(CT):
                for icn in range(4):
                    po = pop.tile([128, 512], dt.float32, tag="po")
                    for et in range(2):
                        nc.tensor.matmul(
                            po,
                            wo_sb[:, et, ct * 128 : (ct + 1) * 128],
                            outn[et][:, icn * 512 : (icn + 1) * 512],
                            start=(et == 0), stop=(et == 1),
                        )
                    ot = otp.tile([128, 512], dt.float32, tag="ot")
                    nc.vector.tensor_copy(out=ot, in_=po)
                    nc.sync.dma_start(
                        out=out_d[ct * 128 : (ct + 1) * 128,
                                  icn * 512 : (icn + 1) * 512],
                        in_=ot,
                    )


def _prep_inputs(x, mask, gamma_ln, gamma_q, gamma_k, Wq, Wkv, Wo):
    x = np.asarray(x, dtype=np.float32)
    mask = np.asarray(mask)
    gamma_ln = np.asarray(gamma_ln, dtype=np.float32)
    gamma_q = np.asarray(gamma_q, dtype=np.float32)
    gamma_k = np.asarray(gamma_k, dtype=np.float32)
    Wq = np.asarray(Wq, dtype=np.float32)
    Wkv = np.asarray(Wkv, dtype=np.float32)
    Wo = np.asarray(Wo, dtype=np.float32)

    Wqg = np.ascontiguousarray(Wq * gamma_ln[:, None])
    Wk = np.ascontiguousarray(Wkv[:, :DIM] * gamma_ln[:, None])
    Wv = np.ascontiguousarray(Wkv[:, DIM:] * gamma_ln[:, None])
    mb = np.where(mask, np.float32(0.0), np.float32(NEG)).astype(np.float32)

    sq = np.sqrt(float(D)).astype(np.float32) if hasattr(np.sqrt(float(D)), "astype") else np.float32(np.sqrt(D))
    gq_full = (np.float32(np.sqrt(D)) * gamma_q.reshape(HEADS, D)).astype(np.float32)
    gk_full = (np.float32(np.sqrt(D)) * gamma_k.reshape(HEADS, D)).astype(np.float32)

    in_maps = []
    for core in range(8):
        b, g = divmod(core, G)
        sl = slice(g * E, (g + 1) * E)
        gq = gq_full[g * HPG : (g + 1) * HPG].reshape(E)
        gk = gk_full[g * HPG : (g + 1) * HPG].reshape(E)
        in_maps.append({
            "x": np.ascontiguousarray(x[b]),
            "mb": np.ascontiguousarray(mb[b].reshape(NT, 128).T),
            "wq": np.ascontiguousarray(Wqg[:, sl]),
            "wk": np.ascontiguousarray(Wk[:, sl]),
            "wv": np.ascontiguousarray(Wv[:, sl]),
            "wo": np.ascontiguousarray(Wo[sl, :]),
            "gq": np.ascontiguousarray(gq.reshape(2, 128).T),
            "gk": np.ascontiguousarray(gk.reshape(2, 128).T),
        })
    return in_maps


def kernel(x, mask, gamma_ln, gamma_q, gamma_k, Wq, Wkv, Wo, _trace=False):
    if "nc" not in _CACHE:
        _CACHE["nc"] = _build_nc()
    nc = _CACHE["nc"]
    in_maps = _prep_inputs(x, mask, gamma_ln, gamma_q, gamma_k, Wq, Wkv, Wo)
    try:
        res = run_bass_kernel_spmd(nc, in_maps, core_ids=list(range(8)),
                                   trace=_trace)
    except Exception:
        # axon terminals occasionally surface transient device errors from
        # earlier sessions; one retry on a fresh attempt is reliable
        res = run_bass_kernel_spmd(nc, in_maps, core_ids=list(range(8)),
                                   trace=_trace)
    _CACHE["last_result"] = res
    out = np.zeros((B, N, DIM), dtype=np.float32)
    for core in range(8):
        b = core // G
        out[b] += res.results[core]["outT"].T
    return out



# revision 29
# speedup vs baseline: 1.2863x; 1.2863x over previous
"""Fused attention kernel for trn2, 8 NeuronCores — v4 (mask-packed keys).

Problem: nn_Attention (b=2, n=2048, dim=1024, heads=16, dim_head=64).
  y = ((softmax(mask(qn @ kn^T)) @ v) @ Wo)   with LN(x) input, qk-RMS-norm.

Sharding: 8 cores = 2 batches x 4 head-groups (4 heads each).

Key ideas vs the original baseline:
  * Key packing: the boolean mask keeps ~50% of keys. Host packs the valid
    keys (sorted) to the front, pads to JN=1280 with copies of row 0 (always
    valid; padded columns get NEG mask bias -> probs exactly 0).
  * LN rstd cancellation: q/k pass through an RMS-norm, so the per-row LN
    scale cancels exactly — q/k projections consume mean-centered x only.
    The v path folds rstd into the PSUM->SBUF copy as a per-partition scalar.
  * No Sqrt/Ln activations at all (they live in different ACT tables than
    Exp and would force a 1.3us table reload around every attention step):
    all rsqrt needs use a DVE integer bit-trick + 2 Newton steps, operating
    on a PE-transposed sumsq layout [n-part, head] whose matmuls cost ~4
    cycles (output free dim = 4).
  * PE transposes use a bf16 identity (1.0 cycles/row vs 1.5 for f32r).
  * probs (exp), v, attn-out, Wo, output are bf16; q/k path stays f32r.
  * Engine balance: LN applies and q2c on GPSIMD(Pool), PSUM->SBUF copies
    alternate ACT/DVE, exp on ACT (sole table -> one load).
  * Software-pipelined emission (queues are in-order): chunk i+1's
    LN/transpose/q-proj is emitted before chunk i's attention j-loop;
    k-side transposes interleave with k-projection chunks; v-proj is
    emitted before the k-norm DRAM round-trip.
"""
import numpy as np
import ml_dtypes

import concourse.bass as bass
import concourse.mybir as mybir
import concourse.tile as tile
from concourse import bacc
from concourse.bass_utils import run_bass_kernel_spmd
from concourse.masks import make_identity

dt = mybir.dt
AF = mybir.ActivationFunctionType
ALU = mybir.AluOpType

B, N, DIM = 2, 2048, 1024
HEADS, D = 16, 64
G = 4            # head groups (cores per batch)
HPG = 4          # heads per group
E = HPG * D      # 256 cols per group
CT = DIM // 128  # 8 contraction tiles
JT = 9           # packed key tiles
JN = JT * 128    # 1280 packed keys
NEG = -1.0e30
RSQC = 0x5F3759DF

_CACHE: dict = {}

KCHUNKS = ((0, 0, 512), (1, 512, 512), (2, 1024, 128))


def _build_nc():
    nc = bacc.Bacc()
    x_d = nc.dram_tensor("x", [N, DIM], dt.float32, kind="ExternalInput")
    xp_d = nc.dram_tensor("xp", [JN, DIM], dt.float32, kind="ExternalInput")
    mbk_d = nc.dram_tensor("mbk", [128, JT], dt.float32, kind="ExternalInput")
    wq_d = nc.dram_tensor("wq", [DIM, E], dt.float32, kind="ExternalInput")
    wk_d = nc.dram_tensor("wk", [DIM, E], dt.float32, kind="ExternalInput")
    wv_d = nc.dram_tensor("wv", [DIM, E], dt.float32, kind="ExternalInput")
    wo_d = nc.dram_tensor("wo", [E, DIM], dt.bfloat16, kind="ExternalInput")
    gq_d = nc.dram_tensor("gq", [128, 2], dt.float32, kind="ExternalInput")
    gk_d = nc.dram_tensor("gk", [128, 2], dt.float32, kind="ExternalInput")
    gq2i_d = nc.dram_tensor("gq2i", [128, 2], dt.float32, kind="ExternalInput")
    gk2i_d = nc.dram_tensor("gk2i", [128, 2], dt.float32, kind="ExternalInput")
    out_d = nc.dram_tensor("outT", [DIM, N], dt.bfloat16, kind="ExternalOutput")

    with tile.TileContext(nc, pool_alloc_mode="queue") as tc:
        _emit(nc, tc, x_d, xp_d, mbk_d, wq_d, wk_d, wv_d, wo_d,
              gq_d, gk_d, gq2i_d, gk2i_d, out_d)
    nc.compile()
    return nc


def _emit(nc, tc, x_d, xp_d, mbk_d, wq_d, wk_d, wv_d, wo_d,
          gq_d, gk_d, gq2i_d, gk2i_d, out_d):
    from contextlib import ExitStack

    ctx = ExitStack()
    with ctx:
        const = ctx.enter_context(tc.tile_pool(name="const", bufs=1))
        drams = ctx.enter_context(tc.tile_pool(name="drams", bufs=1, space="DRAM"))

        # ---- constants ----
        ident_f = const.tile([128, 128], dt.float32, tag="identf")
        make_identity(nc, ident_f)
        identb = const.tile([128, 128], dt.float32r, tag="identb")
        nc.vector.tensor_copy(identb, ident_f)

        ones_f = const.tile([128, 4], dt.float32, tag="onesf")
        nc.vector.memset(ones_f, 1.0)
        ones_bf = const.tile([128, 4], dt.bfloat16, tag="onesb")
        nc.vector.tensor_copy(ones_bf, ones_f)

        blk = []
        for et in range(2):
            bf = const.tile([128, 4], dt.float32, tag=f"blkf{et}", name=f"blkf{et}")
            nc.vector.memset(bf, 0.0)
            nc.vector.memset(bf[0:64, 2 * et : 2 * et + 1], 1.0)
            nc.vector.memset(bf[64:128, 2 * et + 1 : 2 * et + 2], 1.0)
            br = const.tile([128, 4], dt.float32r, tag=f"blk{et}", name=f"blk{et}")
            nc.vector.tensor_copy(br, bf)
            blk.append(br)


        mbk_sb = const.tile([128, JT], dt.float32, tag="mbk")
        gq_sb = const.tile([128, 2], dt.float32, tag="gq")
        gk_sb = const.tile([128, 2], dt.float32, tag="gk")
        gq2i_sb = const.tile([128, 2], dt.float32, tag="gq2i")
        gk2i_sb = const.tile([128, 2], dt.float32, tag="gk2i")

        # ---- persistent activations ----
        pers = ctx.enter_context(tc.tile_pool(name="pers", bufs=1))
        wts_pool = ctx.enter_context(tc.tile_pool(name="wts", bufs=1))
        xntp = ctx.enter_context(tc.tile_pool(name="xntp", bufs=2))
        kn = [pers.tile([128, JN], dt.float32r, tag=f"kn{et}", name=f"kn{et}")
              for et in range(2)]
        v_sb = [pers.tile([128, HPG, 65], dt.bfloat16, tag=f"v{c}", name=f"v{c}")
                for c in range(JT)]

        qnp = ctx.enter_context(tc.tile_pool(name="qnp", bufs=3))
        onp = ctx.enter_context(tc.tile_pool(name="onp", bufs=2))
        rsqp = ctx.enter_context(tc.tile_pool(name="rsqp", bufs=3))

        rstdk_dram = drams.tile([4, JN], dt.float32, tag="rstdk")
        rstdq_dram = drams.tile([4, N], dt.float32, tag="rstdq")
        srp = ctx.enter_context(tc.tile_pool(name="srp", bufs=3))

        def emit_rsqrt(dst, src_ap, w, p=128):
            """dst[p, w] f32 (SBUF) = 1/sqrt(src_ap [p, w] f32).

            Quake bit-trick seed + 2 Newton iterations, all on DVE int/f32
            ALU ops (no ACT table functions)."""
            ti = rsqp.tile([p, w], dt.int32, tag=f"rsq_i{p}_{w}",
                           name=f"rsqi_{p}_{w}")
            nc.vector.tensor_scalar(
                out=ti, in0=src_ap.bitcast(dt.int32), scalar1=1, scalar2=None,
                op0=ALU.logical_shift_right)
            nc.vector.tensor_scalar(
                out=ti, in0=ti, scalar1=-1, scalar2=RSQC,
                op0=ALU.mult, op1=ALU.add)
            y = ti.bitcast(dt.float32)
            u = rsqp.tile([p, w], dt.float32, tag=f"rsq_u{p}_{w}",
                          name=f"rsqu_{p}_{w}")
            for _ in range(2):
                nc.vector.tensor_mul(out=u, in0=y, in1=y)
                nc.vector.scalar_tensor_tensor(
                    out=u, in0=u, scalar=-0.5, in1=src_ap,
                    op0=ALU.mult, op1=ALU.mult)
                nc.vector.scalar_tensor_tensor(
                    out=dst, in0=u, scalar=1.5, in1=y,
                    op0=ALU.add, op1=ALU.mult)
                y = dst
            return dst

        # ================= Phase B: packed keys -> kn, v =================
        bctx = ExitStack()
        with bctx:
            xbp = bctx.enter_context(tc.tile_pool(name="xbp", bufs=2))
            xknp = bctx.enter_context(tc.tile_pool(name="xknp", bufs=5))
            stp = bctx.enter_context(tc.tile_pool(name="stp", bufs=4))
            rbp = bctx.enter_context(tc.tile_pool(name="rbp", bufs=2))
            q2p = bctx.enter_context(tc.tile_pool(name="q2p", bufs=2))
            trB = bctx.enter_context(tc.tile_pool(name="trB", bufs=2, space="PSUM"))
            pskp = bctx.enter_context(tc.tile_pool(name="pskp", bufs=2, space="PSUM"))
            sskp = bctx.enter_context(tc.tile_pool(name="sskp", bufs=2, space="PSUM"))
            psvp = bctx.enter_context(tc.tile_pool(name="psvp", bufs=2, space="PSUM"))
            xkT_pool = bctx.enter_context(tc.tile_pool(name="xkT", bufs=1))
            xkT = [xkT_pool.tile([128, JN], dt.float32r, tag=f"xkT{ct}",
                                 name=f"xkT{ct}") for ct in range(CT)]

            # input loads: first group per-tile (fast availability), then
            # grouped; interleave const/weight DMAs by first-use time
            xg = []
            xt0 = xbp.tile([128, 4, DIM], dt.float32, tag="xt", name="xtB0")
            for k in range(4):
                nc.sync.dma_start(
                    out=xt0[:, k, :],
                    in_=xp_d[k * 128 : (k + 1) * 128, :])
                if k == 1:
                    nc.sync.dma_start(out=gk_sb, in_=gk_d[:, :])
                    nc.sync.dma_start(out=gk2i_sb, in_=gk2i_d[:, :])
            xg.append(xt0)
            nc.sync.dma_start(out=gq_sb, in_=gq_d[:, :])
            nc.sync.dma_start(out=gq2i_sb, in_=gq2i_d[:, :])
            nc.sync.dma_start(out=mbk_sb, in_=mbk_d[:, :])
            for gi, ntiles in ((1, 4), (2, 1)):
                xt = xbp.tile([128, 4, DIM], dt.float32, tag="xt",
                              name=f"xtB{gi}")
                nc.sync.dma_start(
                    out=xt[:, 0:ntiles, :],
                    in_=bass.AP(
                        tensor=xp_d,
                        offset=gi * 4 * 128 * DIM,
                        ap=[[DIM, 128], [128 * DIM, ntiles], [1, DIM]],
                    ),
                )
                xg.append(xt)
            wk_sb = wts_pool.tile([128, CT, E], dt.float32r, tag="wk")
            nc.gpsimd.dma_start(
                out=wk_sb,
                in_=wk_d.rearrange("(ct p) e -> p ct e", p=128).bitcast(dt.float32r))

            # LN: mean-subtract; collect var+eps for the v-path rstd
            varc = const.tile([128, JT], dt.float32, tag="varc")
            xkn = []
            for c in range(JT):
                xt = xg[c // 4][:, c % 4, :]
                st = stp.tile([128, 2, 6], dt.float32, tag="st")
                nc.vector.bn_stats(out=st[:, 0, :], in_=xt[:, 0:512])
                nc.vector.bn_stats(out=st[:, 1, :], in_=xt[:, 512:1024])
                mv = stp.tile([128, 3], dt.float32, tag="mv")
                nc.vector.bn_aggr(out=mv[:, 0:2], in_=st)
                nc.vector.tensor_scalar_add(
                    out=varc[:, c : c + 1], in0=mv[:, 1:2], scalar1=1e-5)
                nc.vector.tensor_scalar_mul(
                    out=mv[:, 2:3], in0=mv[:, 0:1], scalar1=-1.0)
                xnt = xknp.tile([128, DIM], dt.float32r, tag="xkn")
                if c % 2 == 0:
                    nc.vector.tensor_scalar_add(
                        out=xnt, in0=xt, scalar1=mv[:, 2:3])
                else:
                    nc.scalar.activation(xnt, xt, AF.Identity,
                                         bias=mv[:, 2:3])
                xkn.append(xnt)

            # interleaved per chunk: transposes -> k-proj -> sumsq -> rsqrt
            # chain (kn chunks become ready progressively)
            nco = 0
            for ci, c0, w in KCHUNKS:
                for ct in range(CT):
                    bank = trB.tile([128, 512], dt.float32r, tag="bank")
                    for k in range(w // 128):
                        nc.tensor.transpose(
                            out=bank[:, k * 128 : (k + 1) * 128],
                            in_=xkn[ci * 4 + k][:, ct * 128 : (ct + 1) * 128],
                            identity=identb,
                        )
                    if nco % 2 == 0:
                        nc.scalar.copy(
                            out=xkT[ct][:, c0 : c0 + w], in_=bank[:, 0:w])
                    else:
                        nc.vector.tensor_copy(
                            out=xkT[ct][:, c0 : c0 + w], in_=bank[:, 0:w])
                    nco += 1
                q2cs = []
                for et in range(2):
                    psk = pskp.tile([128, 512], dt.float32, tag="psk")
                    for ct in range(CT):
                        nc.tensor.matmul(
                            psk[:, 0:w],
                            wk_sb[:, ct, et * 128 : (et + 1) * 128],
                            xkT[ct][:, c0 : c0 + w],
                            start=(ct == 0), stop=(ct == CT - 1),
                        )
                    nc.vector.tensor_scalar_mul(
                        out=kn[et][:, c0 : c0 + w], in0=psk[:, 0:w],
                        scalar1=gk_sb[:, et : et + 1],
                    )
                    q2c = q2p.tile([128, 512], dt.float32r, tag="q2c")
                    nc.vector.scalar_tensor_tensor(
                        out=q2c[:, 0:w], in0=kn[et][:, c0 : c0 + w],
                        scalar=gk2i_sb[:, et : et + 1],
                        in1=kn[et][:, c0 : c0 + w],
                        op0=ALU.mult, op1=ALU.mult)
                    q2cs.append(q2c)
                ns = w // 128
                ssk_t = sskp.tile([128, 4, 4], dt.float32, tag="sskT",
                                  name=f"sskT{ci}")
                for s in range(ns):
                    for et in range(2):
                        nc.tensor.matmul(
                            ssk_t[:, s, :],
                            q2cs[et][:, s * 128 : (s + 1) * 128],
                            blk[et],
                            start=(s == 0 and et == 0),
                            stop=(s == ns - 1 and et == 1),
                            skip_group_check=True,
                        )
                ssk_s = rbp.tile([128, 4, 4], dt.float32r, tag="ssks")
                nc.vector.tensor_copy(
                    out=ssk_s[:, 0:ns, :], in_=ssk_t[:, 0:ns, :])
                trb2 = trB.tile([128, 512], dt.float32r, tag="bank",
                                name=f"trbsr{ci}")
                nc.tensor.transpose(
                    out=trb2[0 : ns * 4, 0:128],
                    in_=ssk_s.rearrange("p a b -> p (a b)")[:, 0 : ns * 4],
                    identity=identb)
                srk = srp.tile([16, 128], dt.float32, tag="srk")
                nc.vector.tensor_copy(
                    out=srk[0 : ns * 4, :],
                    in_=trb2[0 : ns * 4, 0:128].bitcast(dt.float32))
                rskT = srp.tile([16, 128], dt.float32, tag="rskT")
                emit_rsqrt(rskT[0 : ns * 4, :], srk[0 : ns * 4, :], 128,
                           p=ns * 4)
                nc.gpsimd.dma_start(
                    out=bass.AP(
                        tensor=rstdk_dram.tensor,
                        offset=rstdk_dram.offset + c0,
                        ap=[[128, ns], [JN, 4], [1, 128]],
                    ),
                    in_=rskT[0 : ns * 4, :])
                for et in range(2):
                    rb = rbp.tile([128, 512], dt.float32, tag="rb")
                    nc.gpsimd.dma_start(
                        out=rb[:, 0:w],
                        in_=bass.AP(
                            tensor=rstdk_dram.tensor,
                            offset=rstdk_dram.offset + 2 * et * JN + c0,
                            ap=[[JN, 2], [0, 64], [1, w]],
                        ),
                    )
                    sl = kn[et][:, c0 : c0 + w]
                    nc.vector.tensor_mul(out=sl, in0=sl, in1=rb[:, 0:w])

            vstd = const.tile([128, JT], dt.float32, tag="vstd")
            emit_rsqrt(vstd, varc, JT)
            wv_sb = wts_pool.tile([128, CT, E], dt.float32r, tag="wv")
            nc.gpsimd.dma_start(
                out=wv_sb,
                in_=wv_d.rearrange("(ct p) e -> p ct e", p=128).bitcast(dt.float32r))
            wq_sb = wts_pool.tile([128, CT, E], dt.float32r, tag="wq")
            nc.gpsimd.dma_start(
                out=wq_sb,
                in_=wq_d.rearrange("(ct p) e -> p ct e", p=128).bitcast(dt.float32r))
            # v projection
            for c in range(JT):
                psv = psvp.tile([128, E], dt.float32, tag="psv")
                for ct in range(CT):
                    nc.tensor.matmul(
                        psv,
                        xkT[ct][:, c * 128 : (c + 1) * 128],
                        wv_sb[:, ct, :],
                        start=(ct == 0), stop=(ct == CT - 1),
                    )
                nc.vector.tensor_scalar_mul(
                    out=v_sb[c][:, :, 0:64],
                    in0=psv.rearrange("p (h d) -> p h d", d=64),
                    scalar1=vstd[:, c : c + 1],
                )
                nc.vector.tensor_copy(out=v_sb[c][:, :, 64], in_=ones_bf)

            wo_sb = wts_pool.tile([128, 2, DIM], dt.bfloat16, tag="wo")
            nc.gpsimd.dma_start(out=wo_sb,
                              in_=wo_d.rearrange("(et p) c -> p et c", p=128))

        # ================= Phase C: query chunks =================
        xqp = ctx.enter_context(tc.tile_pool(name="xqp", bufs=2))
        xqnp = ctx.enter_context(tc.tile_pool(name="xqnp", bufs=8))
        stq = ctx.enter_context(tc.tile_pool(name="stq", bufs=4))
        rbq = ctx.enter_context(tc.tile_pool(name="rbq", bufs=2))
        q2q = ctx.enter_context(tc.tile_pool(name="q2q", bufs=2))
        expp = ctx.enter_context(tc.tile_pool(name="expp", bufs=3))
        dnp = ctx.enter_context(tc.tile_pool(name="dnp", bufs=2))
        rdp = ctx.enter_context(tc.tile_pool(name="rdp", bufs=2))
        otp = ctx.enter_context(tc.tile_pool(name="otp", bufs=3))
        trC = ctx.enter_context(tc.tile_pool(name="trC", bufs=1, space="PSUM"))
        ssqp = ctx.enter_context(tc.tile_pool(name="ssqp", bufs=1, space="PSUM"))
        scp = ctx.enter_context(tc.tile_pool(name="scp", bufs=2, space="PSUM"))
        avp = ctx.enter_context(tc.tile_pool(name="avp", bufs=1, space="PSUM"))

        def emit_qside(icn):
            """LN + transpose + q-projection + rms-norm for query chunk icn."""
            c0 = icn * 512
            xnTc = xntp.tile([128, CT, 512], dt.float32r, tag="xnTc",
                             name=f"xnTc{icn}")
            xt = xqp.tile([128, 4, DIM], dt.float32, tag="xt", name=f"xtC{icn}")
            nc.sync.dma_start(
                out=xt,
                in_=bass.AP(
                    tensor=x_d,
                    offset=c0 * DIM,
                    ap=[[DIM, 128], [128 * DIM, 4], [1, DIM]],
                ),
            )
            xqn = []
            for k in range(4):
                xnt = xqnp.tile([128, DIM], dt.float32r, tag="xqn")
                if icn < 2:
                    st = stq.tile([128, 2], dt.float32, tag="sta")
                    nc.scalar.activation(xnt, xt[:, k, :], AF.Copy,
                                         accum_out=st[:, 0:1])
                    nc.vector.tensor_scalar_mul(
                        out=st[:, 1:2], in0=st[:, 0:1], scalar1=-1.0 / DIM)
                    nc.vector.tensor_scalar_add(out=xnt, in0=xnt,
                                                scalar1=st[:, 1:2])
                else:
                    st = stq.tile([128, 2, 6], dt.float32, tag="st")
                    nc.vector.bn_stats(out=st[:, 0, :], in_=xt[:, k, 0:512])
                    nc.vector.bn_stats(out=st[:, 1, :], in_=xt[:, k, 512:1024])
                    mv = stq.tile([128, 3], dt.float32, tag="mv")
                    nc.vector.bn_aggr(out=mv[:, 0:2], in_=st)
                    nc.vector.tensor_scalar_mul(
                        out=mv[:, 2:3], in0=mv[:, 0:1], scalar1=-1.0)
                    if k % 2 == 0:
                        nc.vector.tensor_scalar_add(out=xnt, in0=xt[:, k, :],
                                                    scalar1=mv[:, 2:3])
                    else:
                        nc.scalar.activation(xnt, xt[:, k, :], AF.Identity,
                                             bias=mv[:, 2:3])
                xqn.append(xnt)
            for ct in range(CT):
                bank = trC.tile([128, 512], dt.float32r, tag="bank")
                for k in range(4):
                    nc.tensor.transpose(
                        out=bank[:, k * 128 : (k + 1) * 128],
                        in_=xqn[k][:, ct * 128 : (ct + 1) * 128],
                        identity=identb,
                    )
                nc.vector.tensor_copy(out=xnTc[:, ct, :], in_=bank)

            qts = []
            ssq = ssqp.tile([128, 4, 4], dt.float32, tag="ssq")
            for et in range(2):
                psq = trC.tile([128, 512], dt.float32r, tag="bank",
                               name=f"psq{icn}_{et}").bitcast(dt.float32)
                for ct in range(CT):
                    nc.tensor.matmul(
                        psq,
                        wq_sb[:, ct, et * 128 : (et + 1) * 128],
                        xnTc[:, ct, :],
                        start=(ct == 0), stop=(ct == CT - 1),
                    )
                qt = qnp.tile([128, 512], dt.float32r, tag=f"qn{et}",
                              name=f"qn{et}_{icn}")
                nc.vector.tensor_scalar_mul(
                    out=qt, in0=psq, scalar1=gq_sb[:, et : et + 1])
                q2c = q2q.tile([128, 512], dt.float32r, tag="q2c")
                nc.vector.scalar_tensor_tensor(
                    out=q2c, in0=qt, scalar=gq2i_sb[:, et : et + 1], in1=qt,
                    op0=ALU.mult, op1=ALU.mult)
                for s in range(4):
                    nc.tensor.matmul(
                        ssq[:, s, :],
                        q2c[:, s * 128 : (s + 1) * 128],
                        blk[et],
                        start=(et == 0 and s == 0),
                        stop=(et == 1 and s == 3),
                        skip_group_check=True,
                    )
                qts.append(qt)
            ssq_s = rbq.tile([128, 4, 4], dt.float32r, tag="ssqs")
            nc.vector.tensor_copy(out=ssq_s, in_=ssq)
            trb2 = trC.tile([128, 512], dt.float32r, tag="bank",
                            name=f"trcsr{icn}")
            nc.tensor.transpose(
                out=trb2[0:16, 0:128],
                in_=ssq_s.rearrange("p a b -> p (a b)"),
                identity=identb)
            srq = srp.tile([16, 128], dt.float32, tag="srq")
            nc.vector.tensor_copy(out=srq,
                                  in_=trb2[0:16, 0:128].bitcast(dt.float32))
            rsqT = srp.tile([16, 128], dt.float32, tag="rsqT")
            emit_rsqrt(rsqT, srq, 128, p=16)
            nc.gpsimd.dma_start(
                out=bass.AP(
                    tensor=rstdq_dram.tensor,
                    offset=rstdq_dram.offset + c0,
                    ap=[[128, 4], [N, 4], [1, 128]],
                ),
                in_=rsqT)
            for et in range(2):
                rb = rbq.tile([128, 512], dt.float32, tag="rb")
                nc.gpsimd.dma_start(
                    out=rb,
                    in_=bass.AP(
                        tensor=rstdq_dram.tensor,
                        offset=rstdq_dram.offset + 2 * et * N + c0,
                        ap=[[N, 2], [0, 64], [1, 512]],
                    ),
                )
                nc.vector.tensor_mul(out=qts[et], in0=qts[et], in1=rb)
            return qts

        def emit_outproj(icn, outn):
            c0 = icn * 512
            for ct in range(CT):
                po = trC.tile([128, 512], dt.float32r, tag="bank",
                              name=f"po{icn}_{ct}").bitcast(dt.float32)
                for et in range(2):
                    nc.tensor.matmul(
                        po,
                        wo_sb[:, et, ct * 128 : (ct + 1) * 128],
                        outn[et],
                        start=(et == 0), stop=(et == 1),
                    )
                ott = otp.tile([128, 512], dt.bfloat16, tag="ot")
                nc.vector.tensor_copy(out=ott, in_=po)
                nc.sync.dma_start(
                    out=out_d[ct * 128 : (ct + 1) * 128, c0 : c0 + 512],
                    in_=ott,
                )

        qn_pipe = [emit_qside(0), emit_qside(1)]
        pending = None
        for icn in range(4):
            if icn < 2:
                qn_pipe.append(emit_qside(icn + 2))
            qn_cur = qn_pipe[icn]
            c0 = icn * 512
            outn = []
            for et in range(2):
                if et == 1 and pending is not None:
                    emit_outproj(*pending)
                    pending = None
                avA = avp.tile([65, 512], dt.float32, tag="avA")
                avB = avp.tile([65, 512], dt.float32, tag="avB")

                def emit_score(jt):
                    j0 = jt * 128
                    sc = scp.tile([128, 1024], dt.float32, tag="sc",
                                  name=f"sc{icn}_{et}_{jt}")
                    nc.tensor.matmul(
                        sc[:, 0:512], kn[et][0:64, j0 : j0 + 128],
                        qn_cur[et][0:64, :], start=True, stop=True)
                    nc.tensor.matmul(
                        sc[:, 512:1024], kn[et][64:128, j0 : j0 + 128],
                        qn_cur[et][64:128, :], start=True, stop=True)
                    return sc

                sc_next = emit_score(0)
                for jt in range(JT):
                    sc = sc_next
                    ex = expp.tile([128, 1024], dt.bfloat16, tag="ex")
                    nc.scalar.activation(
                        ex, sc, AF.Exp, bias=mbk_sb[:, jt : jt + 1])
                    if jt + 1 < JT:
                        sc_next = emit_score(jt + 1)
                    st_, sp_ = (jt == 0), (jt == JT - 1)
                    nc.tensor.matmul(
                        avA, v_sb[jt][:, 2 * et, :], ex[:, 0:512],
                        start=st_, stop=sp_, skip_group_check=True)
                    nc.tensor.matmul(
                        avB, v_sb[jt][:, 2 * et + 1, :], ex[:, 512:1024],
                        start=st_, stop=sp_, skip_group_check=True)
                # copy av banks to SBUF immediately (frees the PSUM banks so
                # the next head-pair's accumulation can start), then do the
                # denominator round-trip from the SBUF copy
                avs = dnp.tile([128, 1024], dt.float32, tag="dn")
                nc.vector.tensor_copy(out=avs[0:65, 0:512], in_=avA)
                nc.vector.tensor_copy(out=avs[0:65, 512:1024], in_=avB)
                ddr = drams.tile([2, 512], dt.float32, tag=f"ddr{icn}{et}",
                                 name=f"ddr{icn}{et}")
                nc.gpsimd.dma_start(
                    out=ddr.rearrange("a b -> (a b)")[None, :],
                    in_=avs[64:65, :])
                rbden = rdp.tile([64, 2, 512], dt.float32, tag="rbden")
                for h in range(2):
                    nc.gpsimd.dma_start(
                        out=rbden[:, h, :],
                        in_=bass.AP(
                            tensor=ddr.tensor,
                            offset=ddr.offset + h * 512,
                            ap=[[0, 64], [1, 512]],
                        ),
                    )
                nc.vector.reciprocal_approx_fast(out=rbden, in_=rbden)
                ot = onp.tile([128, 512], dt.bfloat16, tag=f"on{et}",
                              name=f"on{et}_{icn}")
                for h in range(2):
                    nc.vector.tensor_mul(
                        out=ot[h * 64 : (h + 1) * 64, :],
                        in0=avs[0:64, h * 512 : (h + 1) * 512],
                        in1=rbden[:, h, :])
                outn.append(ot)
            pending = (icn, outn)
        emit_outproj(*pending)


def _prep_inputs(x, mask, gamma_ln, gamma_q, gamma_k, Wq, Wkv, Wo):
    x = np.asarray(x, dtype=np.float32)
    mask = np.asarray(mask)
    gamma_ln = np.asarray(gamma_ln, dtype=np.float32)
    gamma_q = np.asarray(gamma_q, dtype=np.float32)
    gamma_k = np.asarray(gamma_k, dtype=np.float32)
    Wq = np.asarray(Wq, dtype=np.float32)
    Wkv = np.asarray(Wkv, dtype=np.float32)
    Wo = np.asarray(Wo, dtype=np.float32)

    Wqg = np.ascontiguousarray(Wq * gamma_ln[:, None])
    Wk = np.ascontiguousarray(Wkv[:, :DIM] * gamma_ln[:, None])
    Wv = np.ascontiguousarray(Wkv[:, DIM:] * gamma_ln[:, None])

    gq_full = (np.float32(np.sqrt(D)) * gamma_q.reshape(HEADS, D)).astype(np.float32)
    gk_full = (np.float32(np.sqrt(D)) * gamma_k.reshape(HEADS, D)).astype(np.float32)

    packs = []
    for b in range(B):
        valid = np.flatnonzero(mask[b])
        nv = len(valid)
        assert nv <= JN, f"valid keys {nv} > JN={JN}; raise JT"
        idx = np.zeros(JN, np.int64)
        idx[:nv] = valid
        xp = np.ascontiguousarray(x[b][idx])
        mb = np.where(np.arange(JN) < nv, np.float32(0.0),
                      np.float32(NEG)).astype(np.float32)
        mbk = np.ascontiguousarray(mb.reshape(JT, 128).T)
        packs.append((xp, mbk))

    in_maps = []
    for core in range(8):
        b, g = divmod(core, G)
        sl = slice(g * E, (g + 1) * E)
        gq = gq_full[g * HPG : (g + 1) * HPG].reshape(E)
        gk = gk_full[g * HPG : (g + 1) * HPG].reshape(E)
        gq2 = gq.reshape(2, 128).T
        gk2 = gk.reshape(2, 128).T
        xp, mbk = packs[b]
        in_maps.append({
            "x": np.ascontiguousarray(x[b]),
            "xp": xp,
            "mbk": mbk,
            "wq": np.ascontiguousarray(Wqg[:, sl]),
            "wk": np.ascontiguousarray(Wk[:, sl]),
            "wv": np.ascontiguousarray(Wv[:, sl]),
            "wo": np.ascontiguousarray(Wo[sl, :]).astype(ml_dtypes.bfloat16),
            "gq": np.ascontiguousarray(gq2),
            "gk": np.ascontiguousarray(gk2),
            "gq2i": np.ascontiguousarray(1.0 / (gq2 * gq2)),
            "gk2i": np.ascontiguousarray(1.0 / (gk2 * gk2)),
        })
    return in_maps


def kernel(x, mask, gamma_ln, gamma_q, gamma_k, Wq, Wkv, Wo, _trace=False):
    if "nc" not in _CACHE:
        _CACHE["nc"] = _build_nc()
    nc = _CACHE["nc"]
    in_maps = _prep_inputs(x, mask, gamma_ln, gamma_q, gamma_k, Wq, Wkv, Wo)
    try:
        res = run_bass_kernel_spmd(nc, in_maps, core_ids=list(range(8)),
                                   trace=_trace)
    except Exception:
        # axon terminals occasionally surface transient device errors from
        # earlier sessions; one retry on a fresh attempt is reliable
        res = run_bass_kernel_spmd(nc, in_maps, core_ids=list(range(8)),
                                   trace=_trace)
    _CACHE["last_result"] = res
    out = np.zeros((B, N, DIM), dtype=np.float32)
    for core in range(8):
        b = core // G
        out[b] += res.results[core]["outT"].astype(np.float32).T
    return out


# revision 42
# speedup vs baseline: 1.2889x; 1.0020x over previous
"""Fused attention kernel for trn2, 8 NeuronCores — v4 (mask-packed keys).

Problem: nn_Attention (b=2, n=2048, dim=1024, heads=16, dim_head=64).
  y = ((softmax(mask(qn @ kn^T)) @ v) @ Wo)   with LN(x) input, qk-RMS-norm.

Sharding: 8 cores = 2 batches x 4 head-groups (4 heads each).

Key ideas vs the original baseline:
  * Key packing: the boolean mask keeps ~50% of keys. Host packs the valid
    keys (sorted) to the front, pads to JN=1280 with copies of row 0 (always
    valid; padded columns get NEG mask bias -> probs exactly 0).
  * LN rstd cancellation: q/k pass through an RMS-norm, so the per-row LN
    scale cancels exactly — q/k projections consume mean-centered x only.
    The v path folds rstd into the PSUM->SBUF copy as a per-partition scalar.
  * No Sqrt/Ln activations at all (they live in different ACT tables than
    Exp and would force a 1.3us table reload around every attention step):
    all rsqrt needs use a DVE integer bit-trick + 2 Newton steps, operating
    on a PE-transposed sumsq layout [n-part, head] whose matmuls cost ~4
    cycles (output free dim = 4).
  * PE transposes use a bf16 identity (1.0 cycles/row vs 1.5 for f32r).
  * probs (exp), v, attn-out, Wo, output are bf16; q/k path stays f32r.
  * Engine balance: LN applies and q2c on GPSIMD(Pool), PSUM->SBUF copies
    alternate ACT/DVE, exp on ACT (sole table -> one load).
  * Software-pipelined emission (queues are in-order): chunk i+1's
    LN/transpose/q-proj is emitted before chunk i's attention j-loop;
    k-side transposes interleave with k-projection chunks; v-proj is
    emitted before the k-norm DRAM round-trip.
"""
import numpy as np
import ml_dtypes

import concourse.bass as bass
import concourse.mybir as mybir
import concourse.tile as tile
from concourse import bacc
from concourse.bass_utils import run_bass_kernel_spmd
from concourse.masks import make_identity

dt = mybir.dt
AF = mybir.ActivationFunctionType
ALU = mybir.AluOpType

B, N, DIM = 2, 2048, 1024
HEADS, D = 16, 64
G = 4            # head groups (cores per batch)
HPG = 4          # heads per group
E = HPG * D      # 256 cols per group
CT = DIM // 128  # 8 contraction tiles
JT = 9           # packed key tiles
JN = JT * 128    # 1280 packed keys
NEG = -1.0e30
RSQC = 0x5F3759DF

_CACHE: dict = {}

KCHUNKS = ((0, 0, 512), (1, 512, 512), (2, 1024, 128))


def _build_nc():
    nc = bacc.Bacc()
    x_d = nc.dram_tensor("x", [N, DIM], dt.float32, kind="ExternalInput")
    xp_d = nc.dram_tensor("xp", [JN, DIM], dt.float32, kind="ExternalInput")
    mbk_d = nc.dram_tensor("mbk", [128, JT], dt.float32, kind="ExternalInput")
    wq_d = nc.dram_tensor("wq", [DIM, E], dt.float32, kind="ExternalInput")
    wk_d = nc.dram_tensor("wk", [DIM, E], dt.float32, kind="ExternalInput")
    wv_d = nc.dram_tensor("wv", [DIM, E], dt.float32, kind="ExternalInput")
    wo_d = nc.dram_tensor("wo", [E, DIM], dt.bfloat16, kind="ExternalInput")
    gq_d = nc.dram_tensor("gq", [128, 2], dt.float32, kind="ExternalInput")
    gk_d = nc.dram_tensor("gk", [128, 2], dt.float32, kind="ExternalInput")
    gq2i_d = nc.dram_tensor("gq2i", [128, 2], dt.float32, kind="ExternalInput")
    gk2i_d = nc.dram_tensor("gk2i", [128, 2], dt.float32, kind="ExternalInput")
    out_d = nc.dram_tensor("outT", [DIM, N], dt.bfloat16, kind="ExternalOutput")

    with tile.TileContext(nc, pool_alloc_mode="queue") as tc:
        _emit(nc, tc, x_d, xp_d, mbk_d, wq_d, wk_d, wv_d, wo_d,
              gq_d, gk_d, gq2i_d, gk2i_d, out_d)
    nc.compile()
    return nc


def _emit(nc, tc, x_d, xp_d, mbk_d, wq_d, wk_d, wv_d, wo_d,
          gq_d, gk_d, gq2i_d, gk2i_d, out_d):
    from contextlib import ExitStack

    ctx = ExitStack()
    with ctx:
        const = ctx.enter_context(tc.tile_pool(name="const", bufs=1))
        drams = ctx.enter_context(tc.tile_pool(name="drams", bufs=1, space="DRAM"))

        # ---- constants ----
        ident_f = const.tile([128, 128], dt.float32, tag="identf")
        make_identity(nc, ident_f)
        identb = const.tile([128, 128], dt.float32r, tag="identb")
        nc.vector.tensor_copy(identb, ident_f)

        ones_f = const.tile([128, 4], dt.float32, tag="onesf")
        nc.vector.memset(ones_f, 1.0)
        ones_bf = const.tile([128, 4], dt.bfloat16, tag="onesb")
        nc.vector.tensor_copy(ones_bf, ones_f)

        blk = []
        for et in range(2):
            bf = const.tile([128, 4], dt.float32, tag=f"blkf{et}", name=f"blkf{et}")
            nc.vector.memset(bf, 0.0)
            nc.vector.memset(bf[0:64, 2 * et : 2 * et + 1], 1.0)
            nc.vector.memset(bf[64:128, 2 * et + 1 : 2 * et + 2], 1.0)
            br = const.tile([128, 4], dt.float32r, tag=f"blk{et}", name=f"blk{et}")
            nc.vector.tensor_copy(br, bf)
            blk.append(br)


        mbk_sb = const.tile([128, JT], dt.float32, tag="mbk")
        gq_sb = const.tile([128, 2], dt.float32, tag="gq")
        gk_sb = const.tile([128, 2], dt.float32, tag="gk")
        gq2i_sb = const.tile([128, 2], dt.float32, tag="gq2i")
        gk2i_sb = const.tile([128, 2], dt.float32, tag="gk2i")

        # ---- persistent activations ----
        pers = ctx.enter_context(tc.tile_pool(name="pers", bufs=1))
        wts_pool = ctx.enter_context(tc.tile_pool(name="wts", bufs=1))
        xntp = ctx.enter_context(tc.tile_pool(name="xntp", bufs=2))
        kn = [pers.tile([128, JN], dt.float32r, tag=f"kn{et}", name=f"kn{et}")
              for et in range(2)]
        v_sb = [pers.tile([128, HPG, 65], dt.bfloat16, tag=f"v{c}", name=f"v{c}")
                for c in range(JT)]

        qnp = ctx.enter_context(tc.tile_pool(name="qnp", bufs=3))
        onp = ctx.enter_context(tc.tile_pool(name="onp", bufs=2))
        rsqp = ctx.enter_context(tc.tile_pool(name="rsqp", bufs=3))

        rstdk_dram = drams.tile([4, JN], dt.float32, tag="rstdk")
        rstdq_dram = drams.tile([4, N], dt.float32, tag="rstdq")
        srp = ctx.enter_context(tc.tile_pool(name="srp", bufs=3))

        def emit_rsqrt(dst, src_ap, w, p=128):
            """dst[p, w] f32 (SBUF) = 1/sqrt(src_ap [p, w] f32).

            Quake bit-trick seed + 2 Newton iterations, all on DVE int/f32
            ALU ops (no ACT table functions)."""
            ti = rsqp.tile([p, w], dt.int32, tag=f"rsq_i{p}_{w}",
                           name=f"rsqi_{p}_{w}")
            nc.vector.tensor_scalar(
                out=ti, in0=src_ap.bitcast(dt.int32), scalar1=1, scalar2=None,
                op0=ALU.logical_shift_right)
            nc.vector.tensor_scalar(
                out=ti, in0=ti, scalar1=-1, scalar2=RSQC,
                op0=ALU.mult, op1=ALU.add)
            y = ti.bitcast(dt.float32)
            u = rsqp.tile([p, w], dt.float32, tag=f"rsq_u{p}_{w}",
                          name=f"rsqu_{p}_{w}")
            for _ in range(2):
                nc.vector.tensor_mul(out=u, in0=y, in1=y)
                nc.vector.scalar_tensor_tensor(
                    out=u, in0=u, scalar=-0.5, in1=src_ap,
                    op0=ALU.mult, op1=ALU.mult)
                nc.vector.scalar_tensor_tensor(
                    out=dst, in0=u, scalar=1.5, in1=y,
                    op0=ALU.add, op1=ALU.mult)
                y = dst
            return dst

        # ================= Phase B: packed keys -> kn, v =================
        bctx = ExitStack()
        with bctx:
            xbp = bctx.enter_context(tc.tile_pool(name="xbp", bufs=2))
            xknp = bctx.enter_context(tc.tile_pool(name="xknp", bufs=5))
            stp = bctx.enter_context(tc.tile_pool(name="stp", bufs=4))
            rbp = bctx.enter_context(tc.tile_pool(name="rbp", bufs=2))
            q2p = bctx.enter_context(tc.tile_pool(name="q2p", bufs=2))
            trB = bctx.enter_context(tc.tile_pool(name="trB", bufs=2, space="PSUM"))
            pskp = bctx.enter_context(tc.tile_pool(name="pskp", bufs=2, space="PSUM"))
            sskp = bctx.enter_context(tc.tile_pool(name="sskp", bufs=2, space="PSUM"))
            psvp = bctx.enter_context(tc.tile_pool(name="psvp", bufs=2, space="PSUM"))
            xkT_pool = bctx.enter_context(tc.tile_pool(name="xkT", bufs=1))
            xkT = [xkT_pool.tile([128, JN], dt.float32r, tag=f"xkT{ct}",
                                 name=f"xkT{ct}") for ct in range(CT)]

            # input loads: first group per-tile (fast availability), then
            # grouped; interleave const/weight DMAs by first-use time
            xg = []
            xt0 = xbp.tile([128, 4, DIM], dt.float32, tag="xt", name="xtB0")
            for k in range(4):
                nc.sync.dma_start(
                    out=xt0[:, k, :],
                    in_=xp_d[k * 128 : (k + 1) * 128, :])
                if k == 1:
                    nc.sync.dma_start(out=gk_sb, in_=gk_d[:, :])
                    nc.sync.dma_start(out=gk2i_sb, in_=gk2i_d[:, :])
            xg.append(xt0)
            nc.sync.dma_start(out=gq_sb, in_=gq_d[:, :])
            nc.sync.dma_start(out=gq2i_sb, in_=gq2i_d[:, :])
            nc.sync.dma_start(out=mbk_sb, in_=mbk_d[:, :])
            for gi, ntiles in ((1, 4), (2, 1)):
                xt = xbp.tile([128, 4, DIM], dt.float32, tag="xt",
                              name=f"xtB{gi}")
                nc.sync.dma_start(
                    out=xt[:, 0:ntiles, :],
                    in_=bass.AP(
                        tensor=xp_d,
                        offset=gi * 4 * 128 * DIM,
                        ap=[[DIM, 128], [128 * DIM, ntiles], [1, DIM]],
                    ),
                )
                xg.append(xt)
            wk_sb = wts_pool.tile([128, CT, E], dt.float32r, tag="wk")
            nc.gpsimd.dma_start(
                out=wk_sb,
                in_=wk_d.rearrange("(ct p) e -> p ct e", p=128).bitcast(dt.float32r))

            # LN: mean-subtract; collect var+eps for the v-path rstd
            varc = const.tile([128, JT], dt.float32, tag="varc")
            xkn = []
            for c in range(JT):
                xt = xg[c // 4][:, c % 4, :]
                st = stp.tile([128, 2, 6], dt.float32, tag="st")
                nc.vector.bn_stats(out=st[:, 0, :], in_=xt[:, 0:512])
                nc.vector.bn_stats(out=st[:, 1, :], in_=xt[:, 512:1024])
                mv = stp.tile([128, 3], dt.float32, tag="mv")
                nc.vector.bn_aggr(out=mv[:, 0:2], in_=st)
                nc.vector.tensor_scalar_add(
                    out=varc[:, c : c + 1], in0=mv[:, 1:2], scalar1=1e-5)
                nc.vector.tensor_scalar_mul(
                    out=mv[:, 2:3], in0=mv[:, 0:1], scalar1=-1.0)
                xnt = xknp.tile([128, DIM], dt.float32r, tag="xkn")
                if c % 2 == 0:
                    nc.vector.tensor_scalar_add(
                        out=xnt, in0=xt, scalar1=mv[:, 2:3])
                else:
                    nc.scalar.activation(xnt, xt, AF.Identity,
                                         bias=mv[:, 2:3])
                xkn.append(xnt)

            # interleaved per chunk: transposes -> k-proj -> sumsq -> rsqrt
            # chain (kn chunks become ready progressively)
            nco = 0
            for ci, c0, w in KCHUNKS:
                for ct in range(CT):
                    bank = trB.tile([128, 512], dt.float32r, tag="bank")
                    for k in range(w // 128):
                        nc.tensor.transpose(
                            out=bank[:, k * 128 : (k + 1) * 128],
                            in_=xkn[ci * 4 + k][:, ct * 128 : (ct + 1) * 128],
                            identity=identb,
                        )
                    if nco % 2 == 0:
                        nc.scalar.copy(
                            out=xkT[ct][:, c0 : c0 + w], in_=bank[:, 0:w])
                    else:
                        nc.vector.tensor_copy(
                            out=xkT[ct][:, c0 : c0 + w], in_=bank[:, 0:w])
                    nco += 1
                q2cs = []
                for et in range(2):
                    psk = pskp.tile([128, 512], dt.float32, tag="psk")
                    for ct in range(CT):
                        nc.tensor.matmul(
                            psk[:, 0:w],
                            wk_sb[:, ct, et * 128 : (et + 1) * 128],
                            xkT[ct][:, c0 : c0 + w],
                            start=(ct == 0), stop=(ct == CT - 1),
                        )
                    nc.vector.tensor_scalar_mul(
                        out=kn[et][:, c0 : c0 + w], in0=psk[:, 0:w],
                        scalar1=gk_sb[:, et : et + 1],
                    )
                    q2c = q2p.tile([128, 512], dt.float32r, tag="q2c")
                    nc.vector.scalar_tensor_tensor(
                        out=q2c[:, 0:w], in0=kn[et][:, c0 : c0 + w],
                        scalar=gk2i_sb[:, et : et + 1],
                        in1=kn[et][:, c0 : c0 + w],
                        op0=ALU.mult, op1=ALU.mult)
                    q2cs.append(q2c)
                ns = w // 128
                ssk_t = sskp.tile([128, 4, 4], dt.float32, tag="sskT",
                                  name=f"sskT{ci}")
                for s in range(ns):
                    for et in range(2):
                        nc.tensor.matmul(
                            ssk_t[:, s, :],
                            q2cs[et][:, s * 128 : (s + 1) * 128],
                            blk[et],
                            start=(s == 0 and et == 0),
                            stop=(s == ns - 1 and et == 1),
                            skip_group_check=True,
                        )
                ssk_s = rbp.tile([128, 4, 4], dt.float32r, tag="ssks")
                nc.vector.tensor_copy(
                    out=ssk_s[:, 0:ns, :], in_=ssk_t[:, 0:ns, :])
                trb2 = trB.tile([128, 512], dt.float32r, tag="bank",
                                name=f"trbsr{ci}")
                nc.tensor.transpose(
                    out=trb2[0 : ns * 4, 0:128],
                    in_=ssk_s.rearrange("p a b -> p (a b)")[:, 0 : ns * 4],
                    identity=identb)
                srk = srp.tile([16, 128], dt.float32, tag="srk")
                nc.vector.tensor_copy(
                    out=srk[0 : ns * 4, :],
                    in_=trb2[0 : ns * 4, 0:128].bitcast(dt.float32))
                rskT = srp.tile([16, 128], dt.float32, tag="rskT")
                emit_rsqrt(rskT[0 : ns * 4, :], srk[0 : ns * 4, :], 128,
                           p=ns * 4)
                nc.gpsimd.dma_start(
                    out=bass.AP(
                        tensor=rstdk_dram.tensor,
                        offset=rstdk_dram.offset + c0,
                        ap=[[128, ns], [JN, 4], [1, 128]],
                    ),
                    in_=rskT[0 : ns * 4, :])
                for et in range(2):
                    rb = rbp.tile([128, 512], dt.float32, tag="rb")
                    nc.gpsimd.dma_start(
                        out=rb[:, 0:w],
                        in_=bass.AP(
                            tensor=rstdk_dram.tensor,
                            offset=rstdk_dram.offset + 2 * et * JN + c0,
                            ap=[[JN, 2], [0, 64], [1, w]],
                        ),
                    )
                    sl = kn[et][:, c0 : c0 + w]
                    nc.vector.tensor_mul(out=sl, in0=sl, in1=rb[:, 0:w])

            vstd = const.tile([128, JT], dt.float32, tag="vstd")
            emit_rsqrt(vstd, varc, JT)
            wv_sb = wts_pool.tile([128, CT, E], dt.float32r, tag="wv")
            nc.gpsimd.dma_start(
                out=wv_sb,
                in_=wv_d.rearrange("(ct p) e -> p ct e", p=128).bitcast(dt.float32r))
            wq_sb = wts_pool.tile([128, CT, E], dt.float32r, tag="wq")
            nc.gpsimd.dma_start(
                out=wq_sb,
                in_=wq_d.rearrange("(ct p) e -> p ct e", p=128).bitcast(dt.float32r))
            # v projection
            for c in range(JT):
                psv = psvp.tile([128, E], dt.float32, tag="psv")
                for ct in range(CT):
                    nc.tensor.matmul(
                        psv,
                        xkT[ct][:, c * 128 : (c + 1) * 128],
                        wv_sb[:, ct, :],
                        start=(ct == 0), stop=(ct == CT - 1),
                    )
                nc.vector.tensor_scalar_mul(
                    out=v_sb[c][:, :, 0:64],
                    in0=psv.rearrange("p (h d) -> p h d", d=64),
                    scalar1=vstd[:, c : c + 1],
                )
                nc.vector.tensor_copy(out=v_sb[c][:, :, 64], in_=ones_bf)

            wo_sb = wts_pool.tile([128, 2, DIM], dt.bfloat16, tag="wo")
            nc.gpsimd.dma_start(out=wo_sb,
                              in_=wo_d.rearrange("(et p) c -> p et c", p=128))

        # ================= Phase C: query chunks =================
        xqp = ctx.enter_context(tc.tile_pool(name="xqp", bufs=2))
        xqnp = ctx.enter_context(tc.tile_pool(name="xqnp", bufs=8))
        stq = ctx.enter_context(tc.tile_pool(name="stq", bufs=4))
        rbq = ctx.enter_context(tc.tile_pool(name="rbq", bufs=2))
        q2q = ctx.enter_context(tc.tile_pool(name="q2q", bufs=2))
        expp = ctx.enter_context(tc.tile_pool(name="expp", bufs=3))
        dnp = ctx.enter_context(tc.tile_pool(name="dnp", bufs=2))
        rdp = ctx.enter_context(tc.tile_pool(name="rdp", bufs=2))
        otp = ctx.enter_context(tc.tile_pool(name="otp", bufs=3))
        trC = ctx.enter_context(tc.tile_pool(name="trC", bufs=1, space="PSUM"))
        ssqp = ctx.enter_context(tc.tile_pool(name="ssqp", bufs=1, space="PSUM"))
        scp = ctx.enter_context(tc.tile_pool(name="scp", bufs=2, space="PSUM"))
        avp = ctx.enter_context(tc.tile_pool(name="avp", bufs=1, space="PSUM"))

        def emit_qside(icn):
            """LN + transpose + q-projection + rms-norm for query chunk icn."""
            c0 = icn * 512
            xnTc = xntp.tile([128, CT, 512], dt.float32r, tag="xnTc",
                             name=f"xnTc{icn}")
            xt = xqp.tile([128, 4, DIM], dt.float32, tag="xt", name=f"xtC{icn}")
            nc.sync.dma_start(
                out=xt,
                in_=bass.AP(
                    tensor=x_d,
                    offset=c0 * DIM,
                    ap=[[DIM, 128], [128 * DIM, 4], [1, DIM]],
                ),
            )
            xqn = []
            for k in range(4):
                xnt = xqnp.tile([128, DIM], dt.float32r, tag="xqn")
                if icn < 2:
                    st = stq.tile([128, 2], dt.float32, tag="sta")
                    nc.scalar.activation(xnt, xt[:, k, :], AF.Copy,
                                         accum_out=st[:, 0:1])
                    nc.vector.tensor_scalar_mul(
                        out=st[:, 1:2], in0=st[:, 0:1], scalar1=-1.0 / DIM)
                    nc.vector.tensor_scalar_add(out=xnt, in0=xnt,
                                                scalar1=st[:, 1:2])
                else:
                    st = stq.tile([128, 2, 6], dt.float32, tag="st")
                    nc.vector.bn_stats(out=st[:, 0, :], in_=xt[:, k, 0:512])
                    nc.vector.bn_stats(out=st[:, 1, :], in_=xt[:, k, 512:1024])
                    mv = stq.tile([128, 3], dt.float32, tag="mv")
                    nc.vector.bn_aggr(out=mv[:, 0:2], in_=st)
                    nc.vector.tensor_scalar_mul(
                        out=mv[:, 2:3], in0=mv[:, 0:1], scalar1=-1.0)
                    if k % 2 == 0:
                        nc.vector.tensor_scalar_add(out=xnt, in0=xt[:, k, :],
                                                    scalar1=mv[:, 2:3])
                    else:
                        nc.scalar.activation(xnt, xt[:, k, :], AF.Identity,
                                             bias=mv[:, 2:3])
                xqn.append(xnt)
            for ct in range(CT):
                bank = trC.tile([128, 512], dt.float32r, tag="bank")
                for k in range(4):
                    nc.tensor.transpose(
                        out=bank[:, k * 128 : (k + 1) * 128],
                        in_=xqn[k][:, ct * 128 : (ct + 1) * 128],
                        identity=identb,
                    )
                nc.vector.tensor_copy(out=xnTc[:, ct, :], in_=bank)

            qts = []
            ssq = ssqp.tile([128, 4, 4], dt.float32, tag="ssq")
            for et in range(2):
                psq = trC.tile([128, 512], dt.float32r, tag="bank",
                               name=f"psq{icn}_{et}").bitcast(dt.float32)
                for ct in range(CT):
                    nc.tensor.matmul(
                        psq,
                        wq_sb[:, ct, et * 128 : (et + 1) * 128],
                        xnTc[:, ct, :],
                        start=(ct == 0), stop=(ct == CT - 1),
                    )
                qt = qnp.tile([128, 512], dt.float32r, tag=f"qn{et}",
                              name=f"qn{et}_{icn}")
                nc.vector.tensor_scalar_mul(
                    out=qt, in0=psq, scalar1=gq_sb[:, et : et + 1])
                q2c = q2q.tile([128, 512], dt.float32r, tag="q2c")
                nc.vector.scalar_tensor_tensor(
                    out=q2c, in0=qt, scalar=gq2i_sb[:, et : et + 1], in1=qt,
                    op0=ALU.mult, op1=ALU.mult)
                for s in range(4):
                    nc.tensor.matmul(
                        ssq[:, s, :],
                        q2c[:, s * 128 : (s + 1) * 128],
                        blk[et],
                        start=(et == 0 and s == 0),
                        stop=(et == 1 and s == 3),
                        skip_group_check=True,
                    )
                qts.append(qt)
            ssq_s = rbq.tile([128, 4, 4], dt.float32r, tag="ssqs")
            nc.vector.tensor_copy(out=ssq_s, in_=ssq)
            trb2 = trC.tile([128, 512], dt.float32r, tag="bank",
                            name=f"trcsr{icn}")
            nc.tensor.transpose(
                out=trb2[0:16, 0:128],
                in_=ssq_s.rearrange("p a b -> p (a b)"),
                identity=identb)
            srq = srp.tile([16, 128], dt.float32, tag="srq")
            nc.vector.tensor_copy(out=srq,
                                  in_=trb2[0:16, 0:128].bitcast(dt.float32))
            rsqT = srp.tile([16, 128], dt.float32, tag="rsqT")
            emit_rsqrt(rsqT, srq, 128, p=16)
            nc.sync.dma_start(
                out=bass.AP(
                    tensor=rstdq_dram.tensor,
                    offset=rstdq_dram.offset + c0,
                    ap=[[128, 4], [N, 4], [1, 128]],
                ),
                in_=rsqT)
            for et in range(2):
                rb = rbq.tile([128, 512], dt.float32, tag="rb")
                nc.sync.dma_start(
                    out=rb,
                    in_=bass.AP(
                        tensor=rstdq_dram.tensor,
                        offset=rstdq_dram.offset + 2 * et * N + c0,
                        ap=[[N, 2], [0, 64], [1, 512]],
                    ),
                )
                nc.vector.tensor_mul(out=qts[et], in0=qts[et], in1=rb)
            return qts

        def emit_outproj(icn, outn):
            c0 = icn * 512
            for ct in range(CT):
                po = trC.tile([128, 512], dt.float32r, tag="bank",
                              name=f"po{icn}_{ct}").bitcast(dt.float32)
                for et in range(2):
                    nc.tensor.matmul(
                        po,
                        wo_sb[:, et, ct * 128 : (ct + 1) * 128],
                        outn[et],
                        start=(et == 0), stop=(et == 1),
                    )
                ott = otp.tile([128, 512], dt.bfloat16, tag="ot")
                nc.vector.tensor_copy(out=ott, in_=po)
                nc.sync.dma_start(
                    out=out_d[ct * 128 : (ct + 1) * 128, c0 : c0 + 512],
                    in_=ott,
                )

        qn_pipe = [emit_qside(0), emit_qside(1)]
        pending = None
        for icn in range(4):
            if icn < 2:
                qn_pipe.append(emit_qside(icn + 2))
            qn_cur = qn_pipe[icn]
            c0 = icn * 512
            outn = []
            for et in range(2):
                if et == 1 and pending is not None:
                    emit_outproj(*pending)
                    pending = None
                avA = avp.tile([65, 512], dt.float32, tag="avA")
                avB = avp.tile([65, 512], dt.float32, tag="avB")

                def emit_score(jt):
                    j0 = jt * 128
                    sc = scp.tile([128, 1024], dt.float32, tag="sc",
                                  name=f"sc{icn}_{et}_{jt}")
                    nc.tensor.matmul(
                        sc[:, 0:512], kn[et][0:64, j0 : j0 + 128],
                        qn_cur[et][0:64, :], start=True, stop=True)
                    nc.tensor.matmul(
                        sc[:, 512:1024], kn[et][64:128, j0 : j0 + 128],
                        qn_cur[et][64:128, :], start=True, stop=True)
                    return sc

                sc_next = emit_score(0)
                for jt in range(JT):
                    sc = sc_next
                    ex = expp.tile([128, 1024], dt.bfloat16, tag="ex")
                    nc.scalar.activation(
                        ex, sc, AF.Exp, bias=mbk_sb[:, jt : jt + 1])
                    if jt + 1 < JT:
                        sc_next = emit_score(jt + 1)
                    st_, sp_ = (jt == 0), (jt == JT - 1)
                    nc.tensor.matmul(
                        avA, v_sb[jt][:, 2 * et, :], ex[:, 0:512],
                        start=st_, stop=sp_, skip_group_check=True)
                    nc.tensor.matmul(
                        avB, v_sb[jt][:, 2 * et + 1, :], ex[:, 512:1024],
                        start=st_, stop=sp_, skip_group_check=True)
                # copy av banks to SBUF immediately (frees the PSUM banks so
                # the next head-pair's accumulation can start), then do the
                # denominator round-trip from the SBUF copy
                avs = dnp.tile([128, 1024], dt.float32, tag="dn")
                nc.vector.tensor_copy(out=avs[0:65, 0:512], in_=avA)
                nc.vector.tensor_copy(out=avs[0:65, 512:1024], in_=avB)
                ddr = drams.tile([2, 512], dt.float32, tag=f"ddr{icn}{et}",
                                 name=f"ddr{icn}{et}")
                nc.sync.dma_start(
                    out=ddr.rearrange("a b -> (a b)")[None, :],
                    in_=avs[64:65, :])
                rbden = rdp.tile([64, 2, 512], dt.float32, tag="rbden")
                for h in range(2):
                    nc.sync.dma_start(
                        out=rbden[:, h, :],
                        in_=bass.AP(
                            tensor=ddr.tensor,
                            offset=ddr.offset + h * 512,
                            ap=[[0, 64], [1, 512]],
                        ),
                    )
                nc.vector.reciprocal_approx_fast(out=rbden, in_=rbden)
                ot = onp.tile([128, 512], dt.bfloat16, tag=f"on{et}",
                              name=f"on{et}_{icn}")
                for h in range(2):
                    nc.vector.tensor_mul(
                        out=ot[h * 64 : (h + 1) * 64, :],
                        in0=avs[0:64, h * 512 : (h + 1) * 512],
                        in1=rbden[:, h, :])
                outn.append(ot)
            pending = (icn, outn)
        emit_outproj(*pending)


def _prep_inputs(x, mask, gamma_ln, gamma_q, gamma_k, Wq, Wkv, Wo):
    x = np.asarray(x, dtype=np.float32)
    mask = np.asarray(mask)
    gamma_ln = np.asarray(gamma_ln, dtype=np.float32)
    gamma_q = np.asarray(gamma_q, dtype=np.float32)
    gamma_k = np.asarray(gamma_k, dtype=np.float32)
    Wq = np.asarray(Wq, dtype=np.float32)
    Wkv = np.asarray(Wkv, dtype=np.float32)
    Wo = np.asarray(Wo, dtype=np.float32)

    Wqg = np.ascontiguousarray(Wq * gamma_ln[:, None])
    Wk = np.ascontiguousarray(Wkv[:, :DIM] * gamma_ln[:, None])
    Wv = np.ascontiguousarray(Wkv[:, DIM:] * gamma_ln[:, None])

    gq_full = (np.float32(np.sqrt(D)) * gamma_q.reshape(HEADS, D)).astype(np.float32)
    gk_full = (np.float32(np.sqrt(D)) * gamma_k.reshape(HEADS, D)).astype(np.float32)

    packs = []
    for b in range(B):
        valid = np.flatnonzero(mask[b])
        nv = len(valid)
        assert nv <= JN, f"valid keys {nv} > JN={JN}; raise JT"
        idx = np.zeros(JN, np.int64)
        idx[:nv] = valid
        xp = np.ascontiguousarray(x[b][idx])
        mb = np.where(np.arange(JN) < nv, np.float32(0.0),
                      np.float32(NEG)).astype(np.float32)
        mbk = np.ascontiguousarray(mb.reshape(JT, 128).T)
        packs.append((xp, mbk))

    in_maps = []
    for core in range(8):
        b, g = divmod(core, G)
        sl = slice(g * E, (g + 1) * E)
        gq = gq_full[g * HPG : (g + 1) * HPG].reshape(E)
        gk = gk_full[g * HPG : (g + 1) * HPG].reshape(E)
        gq2 = gq.reshape(2, 128).T
        gk2 = gk.reshape(2, 128).T
        xp, mbk = packs[b]
        in_maps.append({
            "x": np.ascontiguousarray(x[b]),
            "xp": xp,
            "mbk": mbk,
            "wq": np.ascontiguousarray(Wqg[:, sl]),
            "wk": np.ascontiguousarray(Wk[:, sl]),
            "wv": np.ascontiguousarray(Wv[:, sl]),
            "wo": np.ascontiguousarray(Wo[sl, :]).astype(ml_dtypes.bfloat16),
            "gq": np.ascontiguousarray(gq2),
            "gk": np.ascontiguousarray(gk2),
            "gq2i": np.ascontiguousarray(1.0 / (gq2 * gq2)),
            "gk2i": np.ascontiguousarray(1.0 / (gk2 * gk2)),
        })
    return in_maps


def kernel(x, mask, gamma_ln, gamma_q, gamma_k, Wq, Wkv, Wo, _trace=False):
    if "nc" not in _CACHE:
        _CACHE["nc"] = _build_nc()
    nc = _CACHE["nc"]
    in_maps = _prep_inputs(x, mask, gamma_ln, gamma_q, gamma_k, Wq, Wkv, Wo)
    try:
        res = run_bass_kernel_spmd(nc, in_maps, core_ids=list(range(8)),
                                   trace=_trace)
    except Exception:
        # axon terminals occasionally surface transient device errors from
        # earlier sessions; one retry on a fresh attempt is reliable
        res = run_bass_kernel_spmd(nc, in_maps, core_ids=list(range(8)),
                                   trace=_trace)
    _CACHE["last_result"] = res
    out = np.zeros((B, N, DIM), dtype=np.float32)
    for core in range(8):
        b = core // G
        out[b] += res.results[core]["outT"].astype(np.float32).T
    return out


# revision 43
# speedup vs baseline: 1.2961x; 1.0055x over previous
"""Fused attention kernel for trn2, 8 NeuronCores — v4 (mask-packed keys).

Problem: nn_Attention (b=2, n=2048, dim=1024, heads=16, dim_head=64).
  y = ((softmax(mask(qn @ kn^T)) @ v) @ Wo)   with LN(x) input, qk-RMS-norm.

Sharding: 8 cores = 2 batches x 4 head-groups (4 heads each).

Key ideas vs the original baseline:
  * Key packing: the boolean mask keeps ~50% of keys. Host packs the valid
    keys (sorted) to the front, pads to JN=1280 with copies of row 0 (always
    valid; padded columns get NEG mask bias -> probs exactly 0).
  * LN rstd cancellation: q/k pass through an RMS-norm, so the per-row LN
    scale cancels exactly — q/k projections consume mean-centered x only.
    The v path folds rstd into the PSUM->SBUF copy as a per-partition scalar.
  * No Sqrt/Ln activations at all (they live in different ACT tables than
    Exp and would force a 1.3us table reload around every attention step):
    all rsqrt needs use a DVE integer bit-trick + 2 Newton steps, operating
    on a PE-transposed sumsq layout [n-part, head] whose matmuls cost ~4
    cycles (output free dim = 4).
  * PE transposes use a bf16 identity (1.0 cycles/row vs 1.5 for f32r).
  * probs (exp), v, attn-out, Wo, output are bf16; q/k path stays f32r.
  * Engine balance: LN applies and q2c on GPSIMD(Pool), PSUM->SBUF copies
    alternate ACT/DVE, exp on ACT (sole table -> one load).
  * Software-pipelined emission (queues are in-order): chunk i+1's
    LN/transpose/q-proj is emitted before chunk i's attention j-loop;
    k-side transposes interleave with k-projection chunks; v-proj is
    emitted before the k-norm DRAM round-trip.
"""
import numpy as np
import ml_dtypes

import concourse.bass as bass
import concourse.mybir as mybir
import concourse.tile as tile
from concourse import bacc
from concourse.bass_utils import run_bass_kernel_spmd
from concourse.masks import make_identity

dt = mybir.dt
AF = mybir.ActivationFunctionType
ALU = mybir.AluOpType

B, N, DIM = 2, 2048, 1024
HEADS, D = 16, 64
G = 4            # head groups (cores per batch)
HPG = 4          # heads per group
E = HPG * D      # 256 cols per group
CT = DIM // 128  # 8 contraction tiles
JT = 9           # packed key tiles
JN = JT * 128    # 1280 packed keys
NEG = -1.0e30
RSQC = 0x5F3759DF

_CACHE: dict = {}

KCHUNKS = ((0, 0, 512), (1, 512, 512), (2, 1024, 128))


def _build_nc():
    nc = bacc.Bacc()
    x_d = nc.dram_tensor("x", [N, DIM], dt.float32, kind="ExternalInput")
    xp_d = nc.dram_tensor("xp", [JN, DIM], dt.float32, kind="ExternalInput")
    mbk_d = nc.dram_tensor("mbk", [128, JT], dt.float32, kind="ExternalInput")
    wq_d = nc.dram_tensor("wq", [DIM, E], dt.float32, kind="ExternalInput")
    wk_d = nc.dram_tensor("wk", [DIM, E], dt.float32, kind="ExternalInput")
    wv_d = nc.dram_tensor("wv", [DIM, E], dt.float32, kind="ExternalInput")
    wo_d = nc.dram_tensor("wo", [E, DIM], dt.bfloat16, kind="ExternalInput")
    gq_d = nc.dram_tensor("gq", [128, 2], dt.float32, kind="ExternalInput")
    gk_d = nc.dram_tensor("gk", [128, 2], dt.float32, kind="ExternalInput")
    gq2i_d = nc.dram_tensor("gq2i", [128, 2], dt.float32, kind="ExternalInput")
    gk2i_d = nc.dram_tensor("gk2i", [128, 2], dt.float32, kind="ExternalInput")
    out_d = nc.dram_tensor("outT", [DIM, N], dt.bfloat16, kind="ExternalOutput")

    with tile.TileContext(nc, pool_alloc_mode="queue") as tc:
        _emit(nc, tc, x_d, xp_d, mbk_d, wq_d, wk_d, wv_d, wo_d,
              gq_d, gk_d, gq2i_d, gk2i_d, out_d)
    nc.compile()
    return nc


def _emit(nc, tc, x_d, xp_d, mbk_d, wq_d, wk_d, wv_d, wo_d,
          gq_d, gk_d, gq2i_d, gk2i_d, out_d):
    from contextlib import ExitStack

    ctx = ExitStack()
    with ctx:
        const = ctx.enter_context(tc.tile_pool(name="const", bufs=1))
        drams = ctx.enter_context(tc.tile_pool(name="drams", bufs=1, space="DRAM"))

        # ---- constants ----
        ident_f = const.tile([128, 128], dt.float32, tag="identf")
        make_identity(nc, ident_f)
        identb = const.tile([128, 128], dt.float32r, tag="identb")
        nc.vector.tensor_copy(identb, ident_f)

        ones_f = const.tile([128, 4], dt.float32, tag="onesf")
        nc.vector.memset(ones_f, 1.0)
        ones_bf = const.tile([128, 4], dt.bfloat16, tag="onesb")
        nc.vector.tensor_copy(ones_bf, ones_f)

        blk = []
        for et in range(2):
            bf = const.tile([128, 4], dt.float32, tag=f"blkf{et}", name=f"blkf{et}")
            nc.vector.memset(bf, 0.0)
            nc.vector.memset(bf[0:64, 2 * et : 2 * et + 1], 1.0)
            nc.vector.memset(bf[64:128, 2 * et + 1 : 2 * et + 2], 1.0)
            br = const.tile([128, 4], dt.float32r, tag=f"blk{et}", name=f"blk{et}")
            nc.vector.tensor_copy(br, bf)
            blk.append(br)


        mbk_sb = const.tile([128, JT], dt.float32, tag="mbk")
        gq_sb = const.tile([128, 2], dt.float32, tag="gq")
        gk_sb = const.tile([128, 2], dt.float32, tag="gk")
        gq2i_sb = const.tile([128, 2], dt.float32, tag="gq2i")
        gk2i_sb = const.tile([128, 2], dt.float32, tag="gk2i")

        # ---- persistent activations ----
        pers = ctx.enter_context(tc.tile_pool(name="pers", bufs=1))
        wts_pool = ctx.enter_context(tc.tile_pool(name="wts", bufs=1))
        xntp = ctx.enter_context(tc.tile_pool(name="xntp", bufs=2))
        kn = [pers.tile([128, JN], dt.float32r, tag=f"kn{et}", name=f"kn{et}")
              for et in range(2)]
        v_sb = [pers.tile([128, HPG, 65], dt.bfloat16, tag=f"v{c}", name=f"v{c}")
                for c in range(JT)]

        qnp = ctx.enter_context(tc.tile_pool(name="qnp", bufs=3))
        onp = ctx.enter_context(tc.tile_pool(name="onp", bufs=2))
        rsqp = ctx.enter_context(tc.tile_pool(name="rsqp", bufs=3))

        rstdk_dram = drams.tile([4, JN], dt.float32, tag="rstdk")
        rstdq_dram = drams.tile([4, N], dt.float32, tag="rstdq")
        srp = ctx.enter_context(tc.tile_pool(name="srp", bufs=3))

        def emit_rsqrt(dst, src_ap, w, p=128):
            """dst[p, w] f32 (SBUF) = 1/sqrt(src_ap [p, w] f32).

            Quake bit-trick seed + 2 Newton iterations, all on DVE int/f32
            ALU ops (no ACT table functions)."""
            ti = rsqp.tile([p, w], dt.int32, tag=f"rsq_i{p}_{w}",
                           name=f"rsqi_{p}_{w}")
            nc.vector.tensor_scalar(
                out=ti, in0=src_ap.bitcast(dt.int32), scalar1=1, scalar2=None,
                op0=ALU.logical_shift_right)
            nc.vector.tensor_scalar(
                out=ti, in0=ti, scalar1=-1, scalar2=RSQC,
                op0=ALU.mult, op1=ALU.add)
            y = ti.bitcast(dt.float32)
            u = rsqp.tile([p, w], dt.float32, tag=f"rsq_u{p}_{w}",
                          name=f"rsqu_{p}_{w}")
            for _ in range(2):
                nc.vector.tensor_mul(out=u, in0=y, in1=y)
                nc.vector.scalar_tensor_tensor(
                    out=u, in0=u, scalar=-0.5, in1=src_ap,
                    op0=ALU.mult, op1=ALU.mult)
                nc.vector.scalar_tensor_tensor(
                    out=dst, in0=u, scalar=1.5, in1=y,
                    op0=ALU.add, op1=ALU.mult)
                y = dst
            return dst

        # ================= Phase B: packed keys -> kn, v =================
        bctx = ExitStack()
        with bctx:
            xbp = bctx.enter_context(tc.tile_pool(name="xbp", bufs=2))
            xknp = bctx.enter_context(tc.tile_pool(name="xknp", bufs=5))
            stp = bctx.enter_context(tc.tile_pool(name="stp", bufs=4))
            rbp = bctx.enter_context(tc.tile_pool(name="rbp", bufs=2))
            q2p = bctx.enter_context(tc.tile_pool(name="q2p", bufs=2))
            trB = bctx.enter_context(tc.tile_pool(name="trB", bufs=2, space="PSUM"))
            pskp = bctx.enter_context(tc.tile_pool(name="pskp", bufs=2, space="PSUM"))
            sskp = bctx.enter_context(tc.tile_pool(name="sskp", bufs=2, space="PSUM"))
            psvp = bctx.enter_context(tc.tile_pool(name="psvp", bufs=2, space="PSUM"))
            xkT_pool = bctx.enter_context(tc.tile_pool(name="xkT", bufs=1))
            xkT = [xkT_pool.tile([128, JN], dt.float32r, tag=f"xkT{ct}",
                                 name=f"xkT{ct}") for ct in range(CT)]

            # input loads: first group per-tile (fast availability), then
            # grouped; interleave const/weight DMAs by first-use time
            xg = []
            xt0 = xbp.tile([128, 4, DIM], dt.float32, tag="xt", name="xtB0")
            for k in range(4):
                nc.sync.dma_start(
                    out=xt0[:, k, :],
                    in_=xp_d[k * 128 : (k + 1) * 128, :])
                if k == 1:
                    nc.sync.dma_start(out=gk_sb, in_=gk_d[:, :])
                    nc.sync.dma_start(out=gk2i_sb, in_=gk2i_d[:, :])
            xg.append(xt0)
            nc.sync.dma_start(out=gq_sb, in_=gq_d[:, :])
            nc.sync.dma_start(out=gq2i_sb, in_=gq2i_d[:, :])
            nc.sync.dma_start(out=mbk_sb, in_=mbk_d[:, :])
            for gi, ntiles in ((1, 4), (2, 1)):
                xt = xbp.tile([128, 4, DIM], dt.float32, tag="xt",
                              name=f"xtB{gi}")
                nc.sync.dma_start(
                    out=xt[:, 0:ntiles, :],
                    in_=bass.AP(
                        tensor=xp_d,
                        offset=gi * 4 * 128 * DIM,
                        ap=[[DIM, 128], [128 * DIM, ntiles], [1, DIM]],
                    ),
                )
                xg.append(xt)
            wk_sb = wts_pool.tile([128, CT, E], dt.float32r, tag="wk")
            nc.gpsimd.dma_start(
                out=wk_sb,
                in_=wk_d.rearrange("(ct p) e -> p ct e", p=128).bitcast(dt.float32r))

            # LN: mean-subtract; collect var+eps for the v-path rstd
            varc = const.tile([128, JT], dt.float32, tag="varc")
            xkn = []
            for c in range(JT):
                xt = xg[c // 4][:, c % 4, :]
                st = stp.tile([128, 2, 6], dt.float32, tag="st")
                nc.vector.bn_stats(out=st[:, 0, :], in_=xt[:, 0:512])
                nc.vector.bn_stats(out=st[:, 1, :], in_=xt[:, 512:1024])
                mv = stp.tile([128, 3], dt.float32, tag="mv")
                nc.vector.bn_aggr(out=mv[:, 0:2], in_=st)
                nc.vector.tensor_scalar_add(
                    out=varc[:, c : c + 1], in0=mv[:, 1:2], scalar1=1e-5)
                nc.vector.tensor_scalar_mul(
                    out=mv[:, 2:3], in0=mv[:, 0:1], scalar1=-1.0)
                xnt = xknp.tile([128, DIM], dt.float32r, tag="xkn")
                nc.scalar.activation(xnt, xt, AF.Identity, bias=mv[:, 2:3])
                xkn.append(xnt)

            # interleaved per chunk: transposes -> k-proj -> sumsq -> rsqrt
            # chain (kn chunks become ready progressively)
            nco = 0
            for ci, c0, w in KCHUNKS:
                for ct in range(CT):
                    bank = trB.tile([128, 512], dt.float32r, tag="bank")
                    for k in range(w // 128):
                        nc.tensor.transpose(
                            out=bank[:, k * 128 : (k + 1) * 128],
                            in_=xkn[ci * 4 + k][:, ct * 128 : (ct + 1) * 128],
                            identity=identb,
                        )
                    if nco % 2 == 0:
                        nc.scalar.copy(
                            out=xkT[ct][:, c0 : c0 + w], in_=bank[:, 0:w])
                    else:
                        nc.vector.tensor_copy(
                            out=xkT[ct][:, c0 : c0 + w], in_=bank[:, 0:w])
                    nco += 1
                q2cs = []
                for et in range(2):
                    psk = pskp.tile([128, 512], dt.float32, tag="psk")
                    for ct in range(CT):
                        nc.tensor.matmul(
                            psk[:, 0:w],
                            wk_sb[:, ct, et * 128 : (et + 1) * 128],
                            xkT[ct][:, c0 : c0 + w],
                            start=(ct == 0), stop=(ct == CT - 1),
                        )
                    nc.vector.tensor_scalar_mul(
                        out=kn[et][:, c0 : c0 + w], in0=psk[:, 0:w],
                        scalar1=gk_sb[:, et : et + 1],
                    )
                    q2c = q2p.tile([128, 512], dt.float32r, tag="q2c")
                    nc.vector.scalar_tensor_tensor(
                        out=q2c[:, 0:w], in0=kn[et][:, c0 : c0 + w],
                        scalar=gk2i_sb[:, et : et + 1],
                        in1=kn[et][:, c0 : c0 + w],
                        op0=ALU.mult, op1=ALU.mult)
                    q2cs.append(q2c)
                ns = w // 128
                ssk_t = sskp.tile([128, 4, 4], dt.float32, tag="sskT",
                                  name=f"sskT{ci}")
                for s in range(ns):
                    for et in range(2):
                        nc.tensor.matmul(
                            ssk_t[:, s, :],
                            q2cs[et][:, s * 128 : (s + 1) * 128],
                            blk[et],
                            start=(s == 0 and et == 0),
                            stop=(s == ns - 1 and et == 1),
                            skip_group_check=True,
                        )
                ssk_s = rbp.tile([128, 4, 4], dt.float32r, tag="ssks")
                nc.vector.tensor_copy(
                    out=ssk_s[:, 0:ns, :], in_=ssk_t[:, 0:ns, :])
                trb2 = trB.tile([128, 512], dt.float32r, tag="bank",
                                name=f"trbsr{ci}")
                nc.tensor.transpose(
                    out=trb2[0 : ns * 4, 0:128],
                    in_=ssk_s.rearrange("p a b -> p (a b)")[:, 0 : ns * 4],
                    identity=identb)
                srk = srp.tile([16, 128], dt.float32, tag="srk")
                nc.vector.tensor_copy(
                    out=srk[0 : ns * 4, :],
                    in_=trb2[0 : ns * 4, 0:128].bitcast(dt.float32))
                rskT = srp.tile([16, 128], dt.float32, tag="rskT")
                emit_rsqrt(rskT[0 : ns * 4, :], srk[0 : ns * 4, :], 128,
                           p=ns * 4)
                nc.gpsimd.dma_start(
                    out=bass.AP(
                        tensor=rstdk_dram.tensor,
                        offset=rstdk_dram.offset + c0,
                        ap=[[128, ns], [JN, 4], [1, 128]],
                    ),
                    in_=rskT[0 : ns * 4, :])
                for et in range(2):
                    rb = rbp.tile([128, 512], dt.float32, tag="rb")
                    nc.gpsimd.dma_start(
                        out=rb[:, 0:w],
                        in_=bass.AP(
                            tensor=rstdk_dram.tensor,
                            offset=rstdk_dram.offset + 2 * et * JN + c0,
                            ap=[[JN, 2], [0, 64], [1, w]],
                        ),
                    )
                    sl = kn[et][:, c0 : c0 + w]
                    nc.vector.tensor_mul(out=sl, in0=sl, in1=rb[:, 0:w])

            vstd = const.tile([128, JT], dt.float32, tag="vstd")
            emit_rsqrt(vstd, varc, JT)
            wv_sb = wts_pool.tile([128, CT, E], dt.float32r, tag="wv")
            nc.gpsimd.dma_start(
                out=wv_sb,
                in_=wv_d.rearrange("(ct p) e -> p ct e", p=128).bitcast(dt.float32r))
            wq_sb = wts_pool.tile([128, CT, E], dt.float32r, tag="wq")
            nc.gpsimd.dma_start(
                out=wq_sb,
                in_=wq_d.rearrange("(ct p) e -> p ct e", p=128).bitcast(dt.float32r))
            # v projection
            for c in range(JT):
                psv = psvp.tile([128, E], dt.float32, tag="psv")
                for ct in range(CT):
                    nc.tensor.matmul(
                        psv,
                        xkT[ct][:, c * 128 : (c + 1) * 128],
                        wv_sb[:, ct, :],
                        start=(ct == 0), stop=(ct == CT - 1),
                    )
                nc.vector.tensor_scalar_mul(
                    out=v_sb[c][:, :, 0:64],
                    in0=psv.rearrange("p (h d) -> p h d", d=64),
                    scalar1=vstd[:, c : c + 1],
                )
                nc.vector.tensor_copy(out=v_sb[c][:, :, 64], in_=ones_bf)

            wo_sb = wts_pool.tile([128, 2, DIM], dt.bfloat16, tag="wo")
            nc.gpsimd.dma_start(out=wo_sb,
                              in_=wo_d.rearrange("(et p) c -> p et c", p=128))

        # ================= Phase C: query chunks =================
        xqp = ctx.enter_context(tc.tile_pool(name="xqp", bufs=2))
        xqnp = ctx.enter_context(tc.tile_pool(name="xqnp", bufs=8))
        stq = ctx.enter_context(tc.tile_pool(name="stq", bufs=4))
        rbq = ctx.enter_context(tc.tile_pool(name="rbq", bufs=2))
        q2q = ctx.enter_context(tc.tile_pool(name="q2q", bufs=2))
        expp = ctx.enter_context(tc.tile_pool(name="expp", bufs=3))
        dnp = ctx.enter_context(tc.tile_pool(name="dnp", bufs=2))
        rdp = ctx.enter_context(tc.tile_pool(name="rdp", bufs=2))
        otp = ctx.enter_context(tc.tile_pool(name="otp", bufs=3))
        trC = ctx.enter_context(tc.tile_pool(name="trC", bufs=1, space="PSUM"))
        ssqp = ctx.enter_context(tc.tile_pool(name="ssqp", bufs=1, space="PSUM"))
        scp = ctx.enter_context(tc.tile_pool(name="scp", bufs=2, space="PSUM"))
        avp = ctx.enter_context(tc.tile_pool(name="avp", bufs=1, space="PSUM"))

        def emit_qside(icn):
            """LN + transpose + q-projection + rms-norm for query chunk icn."""
            c0 = icn * 512
            xnTc = xntp.tile([128, CT, 512], dt.float32r, tag="xnTc",
                             name=f"xnTc{icn}")
            xt = xqp.tile([128, 4, DIM], dt.float32, tag="xt", name=f"xtC{icn}")
            nc.sync.dma_start(
                out=xt,
                in_=bass.AP(
                    tensor=x_d,
                    offset=c0 * DIM,
                    ap=[[DIM, 128], [128 * DIM, 4], [1, DIM]],
                ),
            )
            xqn = []
            for k in range(4):
                xnt = xqnp.tile([128, DIM], dt.float32r, tag="xqn")
                if icn < 2:
                    st = stq.tile([128, 2], dt.float32, tag="sta")
                    nc.scalar.activation(xnt, xt[:, k, :], AF.Copy,
                                         accum_out=st[:, 0:1])
                    nc.vector.tensor_scalar_mul(
                        out=st[:, 1:2], in0=st[:, 0:1], scalar1=-1.0 / DIM)
                    nc.vector.tensor_scalar_add(out=xnt, in0=xnt,
                                                scalar1=st[:, 1:2])
                else:
                    st = stq.tile([128, 2, 6], dt.float32, tag="st")
                    nc.vector.bn_stats(out=st[:, 0, :], in_=xt[:, k, 0:512])
                    nc.vector.bn_stats(out=st[:, 1, :], in_=xt[:, k, 512:1024])
                    mv = stq.tile([128, 3], dt.float32, tag="mv")
                    nc.vector.bn_aggr(out=mv[:, 0:2], in_=st)
                    nc.vector.tensor_scalar_mul(
                        out=mv[:, 2:3], in0=mv[:, 0:1], scalar1=-1.0)
                    nc.vector.tensor_scalar_add(out=xnt, in0=xt[:, k, :],
                                                scalar1=mv[:, 2:3])
                xqn.append(xnt)
            for ct in range(CT):
                bank = trC.tile([128, 512], dt.float32r, tag="bank")
                for k in range(4):
                    nc.tensor.transpose(
                        out=bank[:, k * 128 : (k + 1) * 128],
                        in_=xqn[k][:, ct * 128 : (ct + 1) * 128],
                        identity=identb,
                    )
                nc.vector.tensor_copy(out=xnTc[:, ct, :], in_=bank)

            qts = []
            ssq = ssqp.tile([128, 4, 4], dt.float32, tag="ssq")
            for et in range(2):
                psq = trC.tile([128, 512], dt.float32r, tag="bank",
                               name=f"psq{icn}_{et}").bitcast(dt.float32)
                for ct in range(CT):
                    nc.tensor.matmul(
                        psq,
                        wq_sb[:, ct, et * 128 : (et + 1) * 128],
                        xnTc[:, ct, :],
                        start=(ct == 0), stop=(ct == CT - 1),
                    )
                qt = qnp.tile([128, 512], dt.float32r, tag=f"qn{et}",
                              name=f"qn{et}_{icn}")
                nc.vector.tensor_scalar_mul(
                    out=qt, in0=psq, scalar1=gq_sb[:, et : et + 1])
                q2c = q2q.tile([128, 512], dt.float32r, tag="q2c")
                nc.vector.scalar_tensor_tensor(
                    out=q2c, in0=qt, scalar=gq2i_sb[:, et : et + 1], in1=qt,
                    op0=ALU.mult, op1=ALU.mult)
                for s in range(4):
                    nc.tensor.matmul(
                        ssq[:, s, :],
                        q2c[:, s * 128 : (s + 1) * 128],
                        blk[et],
                        start=(et == 0 and s == 0),
                        stop=(et == 1 and s == 3),
                        skip_group_check=True,
                    )
                qts.append(qt)
            ssq_s = rbq.tile([128, 4, 4], dt.float32r, tag="ssqs")
            nc.vector.tensor_copy(out=ssq_s, in_=ssq)
            trb2 = trC.tile([128, 512], dt.float32r, tag="bank",
                            name=f"trcsr{icn}")
            nc.tensor.transpose(
                out=trb2[0:16, 0:128],
                in_=ssq_s.rearrange("p a b -> p (a b)"),
                identity=identb)
            srq = srp.tile([16, 128], dt.float32, tag="srq")
            nc.vector.tensor_copy(out=srq,
                                  in_=trb2[0:16, 0:128].bitcast(dt.float32))
            rsqT = srp.tile([16, 128], dt.float32, tag="rsqT")
            emit_rsqrt(rsqT, srq, 128, p=16)
            nc.sync.dma_start(
                out=bass.AP(
                    tensor=rstdq_dram.tensor,
                    offset=rstdq_dram.offset + c0,
                    ap=[[128, 4], [N, 4], [1, 128]],
                ),
                in_=rsqT)
            for et in range(2):
                rb = rbq.tile([128, 512], dt.float32, tag="rb")
                nc.sync.dma_start(
                    out=rb,
                    in_=bass.AP(
                        tensor=rstdq_dram.tensor,
                        offset=rstdq_dram.offset + 2 * et * N + c0,
                        ap=[[N, 2], [0, 64], [1, 512]],
                    ),
                )
                nc.vector.tensor_mul(out=qts[et], in0=qts[et], in1=rb)
            return qts

        def emit_outproj(icn, outn):
            c0 = icn * 512
            for ct in range(CT):
                po = trC.tile([128, 512], dt.float32r, tag="bank",
                              name=f"po{icn}_{ct}").bitcast(dt.float32)
                for et in range(2):
                    nc.tensor.matmul(
                        po,
                        wo_sb[:, et, ct * 128 : (ct + 1) * 128],
                        outn[et],
                        start=(et == 0), stop=(et == 1),
                    )
                ott = otp.tile([128, 512], dt.bfloat16, tag="ot")
                nc.vector.tensor_copy(out=ott, in_=po)
                nc.sync.dma_start(
                    out=out_d[ct * 128 : (ct + 1) * 128, c0 : c0 + 512],
                    in_=ott,
                )

        qn_pipe = [emit_qside(0), emit_qside(1)]
        pending = None
        for icn in range(4):
            if icn < 2:
                qn_pipe.append(emit_qside(icn + 2))
            qn_cur = qn_pipe[icn]
            c0 = icn * 512
            outn = []
            for et in range(2):
                if et == 1 and pending is not None:
                    emit_outproj(*pending)
                    pending = None
                avA = avp.tile([65, 512], dt.float32, tag="avA")
                avB = avp.tile([65, 512], dt.float32, tag="avB")

                def emit_score(jt):
                    j0 = jt * 128
                    sc = scp.tile([128, 1024], dt.float32, tag="sc",
                                  name=f"sc{icn}_{et}_{jt}")
                    nc.tensor.matmul(
                        sc[:, 0:512], kn[et][0:64, j0 : j0 + 128],
                        qn_cur[et][0:64, :], start=True, stop=True)
                    nc.tensor.matmul(
                        sc[:, 512:1024], kn[et][64:128, j0 : j0 + 128],
                        qn_cur[et][64:128, :], start=True, stop=True)
                    return sc

                sc_next = emit_score(0)
                for jt in range(JT):
                    sc = sc_next
                    ex = expp.tile([128, 1024], dt.bfloat16, tag="ex")
                    nc.scalar.activation(
                        ex, sc, AF.Exp, bias=mbk_sb[:, jt : jt + 1])
                    if jt + 1 < JT:
                        sc_next = emit_score(jt + 1)
                    st_, sp_ = (jt == 0), (jt == JT - 1)
                    nc.tensor.matmul(
                        avA, v_sb[jt][:, 2 * et, :], ex[:, 0:512],
                        start=st_, stop=sp_, skip_group_check=True)
                    nc.tensor.matmul(
                        avB, v_sb[jt][:, 2 * et + 1, :], ex[:, 512:1024],
                        start=st_, stop=sp_, skip_group_check=True)
                # copy av banks to SBUF immediately (frees the PSUM banks so
                # the next head-pair's accumulation can start), then do the
                # denominator round-trip from the SBUF copy
                avs = dnp.tile([128, 1024], dt.float32, tag="dn")
                nc.vector.tensor_copy(out=avs[0:65, 0:512], in_=avA)
                nc.vector.tensor_copy(out=avs[0:65, 512:1024], in_=avB)
                ddr = drams.tile([2, 512], dt.float32, tag=f"ddr{icn}{et}",
                                 name=f"ddr{icn}{et}")
                nc.sync.dma_start(
                    out=ddr.rearrange("a b -> (a b)")[None, :],
                    in_=avs[64:65, :])
                rbden = rdp.tile([64, 2, 512], dt.float32, tag="rbden")
                for h in range(2):
                    nc.sync.dma_start(
                        out=rbden[:, h, :],
                        in_=bass.AP(
                            tensor=ddr.tensor,
                            offset=ddr.offset + h * 512,
                            ap=[[0, 64], [1, 512]],
                        ),
                    )
                nc.vector.reciprocal_approx_fast(out=rbden, in_=rbden)
                ot = onp.tile([128, 512], dt.bfloat16, tag=f"on{et}",
                              name=f"on{et}_{icn}")
                for h in range(2):
                    nc.vector.tensor_mul(
                        out=ot[h * 64 : (h + 1) * 64, :],
                        in0=avs[0:64, h * 512 : (h + 1) * 512],
                        in1=rbden[:, h, :])
                outn.append(ot)
            pending = (icn, outn)
        emit_outproj(*pending)


def _prep_inputs(x, mask, gamma_ln, gamma_q, gamma_k, Wq, Wkv, Wo):
    x = np.asarray(x, dtype=np.float32)
    mask = np.asarray(mask)
    gamma_ln = np.asarray(gamma_ln, dtype=np.float32)
    gamma_q = np.asarray(gamma_q, dtype=np.float32)
    gamma_k = np.asarray(gamma_k, dtype=np.float32)
    Wq = np.asarray(Wq, dtype=np.float32)
    Wkv = np.asarray(Wkv, dtype=np.float32)
    Wo = np.asarray(Wo, dtype=np.float32)

    Wqg = np.ascontiguousarray(Wq * gamma_ln[:, None])
    Wk = np.ascontiguousarray(Wkv[:, :DIM] * gamma_ln[:, None])
    Wv = np.ascontiguousarray(Wkv[:, DIM:] * gamma_ln[:, None])

    gq_full = (np.float32(np.sqrt(D)) * gamma_q.reshape(HEADS, D)).astype(np.float32)
    gk_full = (np.float32(np.sqrt(D)) * gamma_k.reshape(HEADS, D)).astype(np.float32)

    packs = []
    for b in range(B):
        valid = np.flatnonzero(mask[b])
        nv = len(valid)
        assert nv <= JN, f"valid keys {nv} > JN={JN}; raise JT"
        idx = np.zeros(JN, np.int64)
        idx[:nv] = valid
        xp = np.ascontiguousarray(x[b][idx])
        mb = np.where(np.arange(JN) < nv, np.float32(0.0),
                      np.float32(NEG)).astype(np.float32)
        mbk = np.ascontiguousarray(mb.reshape(JT, 128).T)
        packs.append((xp, mbk))

    in_maps = []
    for core in range(8):
        b, g = divmod(core, G)
        sl = slice(g * E, (g + 1) * E)
        gq = gq_full[g * HPG : (g + 1) * HPG].reshape(E)
        gk = gk_full[g * HPG : (g + 1) * HPG].reshape(E)
        gq2 = gq.reshape(2, 128).T
        gk2 = gk.reshape(2, 128).T
        xp, mbk = packs[b]
        in_maps.append({
            "x": np.ascontiguousarray(x[b]),
            "xp": xp,
            "mbk": mbk,
            "wq": np.ascontiguousarray(Wqg[:, sl]),
            "wk": np.ascontiguousarray(Wk[:, sl]),
            "wv": np.ascontiguousarray(Wv[:, sl]),
            "wo": np.ascontiguousarray(Wo[sl, :]).astype(ml_dtypes.bfloat16),
            "gq": np.ascontiguousarray(gq2),
            "gk": np.ascontiguousarray(gk2),
            "gq2i": np.ascontiguousarray(1.0 / (gq2 * gq2)),
            "gk2i": np.ascontiguousarray(1.0 / (gk2 * gk2)),
        })
    return in_maps


def kernel(x, mask, gamma_ln, gamma_q, gamma_k, Wq, Wkv, Wo, _trace=False):
    if "nc" not in _CACHE:
        _CACHE["nc"] = _build_nc()
    nc = _CACHE["nc"]
    in_maps = _prep_inputs(x, mask, gamma_ln, gamma_q, gamma_k, Wq, Wkv, Wo)
    try:
        res = run_bass_kernel_spmd(nc, in_maps, core_ids=list(range(8)),
                                   trace=_trace)
    except Exception:
        # axon terminals occasionally surface transient device errors from
        # earlier sessions; one retry on a fresh attempt is reliable
        res = run_bass_kernel_spmd(nc, in_maps, core_ids=list(range(8)),
                                   trace=_trace)
    _CACHE["last_result"] = res
    out = np.zeros((B, N, DIM), dtype=np.float32)
    for core in range(8):
        b = core // G
        out[b] += res.results[core]["outT"].astype(np.float32).T
    return out


# revision 45
# speedup vs baseline: 1.3006x; 1.0035x over previous
"""Fused attention kernel for trn2, 8 NeuronCores — v4 (mask-packed keys).

Problem: nn_Attention (b=2, n=2048, dim=1024, heads=16, dim_head=64).
  y = ((softmax(mask(qn @ kn^T)) @ v) @ Wo)   with LN(x) input, qk-RMS-norm.

Sharding: 8 cores = 2 batches x 4 head-groups (4 heads each).

Key ideas vs the original baseline:
  * Key packing: the boolean mask keeps ~50% of keys. Host packs the valid
    keys (sorted) to the front, pads to JN=1280 with copies of row 0 (always
    valid; padded columns get NEG mask bias -> probs exactly 0).
  * LN rstd cancellation: q/k pass through an RMS-norm, so the per-row LN
    scale cancels exactly — q/k projections consume mean-centered x only.
    The v path folds rstd into the PSUM->SBUF copy as a per-partition scalar.
  * No Sqrt/Ln activations at all (they live in different ACT tables than
    Exp and would force a 1.3us table reload around every attention step):
    all rsqrt needs use a DVE integer bit-trick + 2 Newton steps, operating
    on a PE-transposed sumsq layout [n-part, head] whose matmuls cost ~4
    cycles (output free dim = 4).
  * PE transposes use a bf16 identity (1.0 cycles/row vs 1.5 for f32r).
  * probs (exp), v, attn-out, Wo, output are bf16; q/k path stays f32r.
  * Engine balance: LN applies and q2c on GPSIMD(Pool), PSUM->SBUF copies
    alternate ACT/DVE, exp on ACT (sole table -> one load).
  * Software-pipelined emission (queues are in-order): chunk i+1's
    LN/transpose/q-proj is emitted before chunk i's attention j-loop;
    k-side transposes interleave with k-projection chunks; v-proj is
    emitted before the k-norm DRAM round-trip.
"""
import numpy as np
import ml_dtypes

import concourse.bass as bass
import concourse.mybir as mybir
import concourse.tile as tile
from concourse import bacc
from concourse.bass_utils import run_bass_kernel_spmd
from concourse.masks import make_identity

dt = mybir.dt
AF = mybir.ActivationFunctionType
ALU = mybir.AluOpType

B, N, DIM = 2, 2048, 1024
HEADS, D = 16, 64
G = 4            # head groups (cores per batch)
HPG = 4          # heads per group
E = HPG * D      # 256 cols per group
CT = DIM // 128  # 8 contraction tiles
JT = 9           # packed key tiles
JN = JT * 128    # 1280 packed keys
NEG = -1.0e30
RSQC = 0x5F3759DF

_CACHE: dict = {}

KCHUNKS = ((0, 0, 512), (1, 512, 512), (2, 1024, 128))


def _build_nc():
    nc = bacc.Bacc()
    x_d = nc.dram_tensor("x", [N, DIM], dt.float32, kind="ExternalInput")
    xp_d = nc.dram_tensor("xp", [JN, DIM], dt.float32, kind="ExternalInput")
    mbk_d = nc.dram_tensor("mbk", [128, JT], dt.float32, kind="ExternalInput")
    wq_d = nc.dram_tensor("wq", [DIM, E], dt.float32, kind="ExternalInput")
    wk_d = nc.dram_tensor("wk", [DIM, E], dt.float32, kind="ExternalInput")
    wv_d = nc.dram_tensor("wv", [DIM, E], dt.float32, kind="ExternalInput")
    wo_d = nc.dram_tensor("wo", [E, DIM], dt.bfloat16, kind="ExternalInput")
    gq_d = nc.dram_tensor("gq", [128, 2], dt.float32, kind="ExternalInput")
    gk_d = nc.dram_tensor("gk", [128, 2], dt.float32, kind="ExternalInput")
    gq2i_d = nc.dram_tensor("gq2i", [128, 2], dt.float32, kind="ExternalInput")
    gk2i_d = nc.dram_tensor("gk2i", [128, 2], dt.float32, kind="ExternalInput")
    out_d = nc.dram_tensor("outT", [DIM, N], dt.bfloat16, kind="ExternalOutput")

    with tile.TileContext(nc, pool_alloc_mode="queue") as tc:
        _emit(nc, tc, x_d, xp_d, mbk_d, wq_d, wk_d, wv_d, wo_d,
              gq_d, gk_d, gq2i_d, gk2i_d, out_d)
    nc.compile()
    return nc


def _emit(nc, tc, x_d, xp_d, mbk_d, wq_d, wk_d, wv_d, wo_d,
          gq_d, gk_d, gq2i_d, gk2i_d, out_d):
    from contextlib import ExitStack

    ctx = ExitStack()
    with ctx:
        const = ctx.enter_context(tc.tile_pool(name="const", bufs=1))
        drams = ctx.enter_context(tc.tile_pool(name="drams", bufs=1, space="DRAM"))

        # ---- constants ----
        ident_f = const.tile([128, 128], dt.float32, tag="identf")
        make_identity(nc, ident_f)
        identb = const.tile([128, 128], dt.float32r, tag="identb")
        nc.vector.tensor_copy(identb, ident_f)

        ones_f = const.tile([128, 4], dt.float32, tag="onesf")
        nc.vector.memset(ones_f, 1.0)
        ones_bf = const.tile([128, 4], dt.bfloat16, tag="onesb")
        nc.vector.tensor_copy(ones_bf, ones_f)

        blk = []
        for et in range(2):
            bf = const.tile([128, 4], dt.float32, tag=f"blkf{et}", name=f"blkf{et}")
            nc.vector.memset(bf, 0.0)
            nc.vector.memset(bf[0:64, 2 * et : 2 * et + 1], 1.0)
            nc.vector.memset(bf[64:128, 2 * et + 1 : 2 * et + 2], 1.0)
            br = const.tile([128, 4], dt.float32r, tag=f"blk{et}", name=f"blk{et}")
            nc.vector.tensor_copy(br, bf)
            blk.append(br)


        mbk_sb = const.tile([128, JT], dt.float32, tag="mbk")
        gq_sb = const.tile([128, 2], dt.float32, tag="gq")
        gk_sb = const.tile([128, 2], dt.float32, tag="gk")
        gq2i_sb = const.tile([128, 2], dt.float32, tag="gq2i")
        gk2i_sb = const.tile([128, 2], dt.float32, tag="gk2i")

        # ---- persistent activations ----
        pers = ctx.enter_context(tc.tile_pool(name="pers", bufs=1))
        wts_pool = ctx.enter_context(tc.tile_pool(name="wts", bufs=1))
        xntp = ctx.enter_context(tc.tile_pool(name="xntp", bufs=2))
        kn = [pers.tile([128, JN], dt.float32r, tag=f"kn{et}", name=f"kn{et}")
              for et in range(2)]
        v_sb = [pers.tile([128, HPG, 65], dt.bfloat16, tag=f"v{c}", name=f"v{c}")
                for c in range(JT)]

        qnp = ctx.enter_context(tc.tile_pool(name="qnp", bufs=3))
        onp = ctx.enter_context(tc.tile_pool(name="onp", bufs=2))
        rsqp = ctx.enter_context(tc.tile_pool(name="rsqp", bufs=3))

        rstdk_dram = drams.tile([4, JN], dt.float32, tag="rstdk")
        rstdq_dram = drams.tile([4, N], dt.float32, tag="rstdq")
        srp = ctx.enter_context(tc.tile_pool(name="srp", bufs=3))

        def emit_rsqrt(dst, src_ap, w, p=128):
            """dst[p, w] f32 (SBUF) = 1/sqrt(src_ap [p, w] f32).

            Quake bit-trick seed + 2 Newton iterations, all on DVE int/f32
            ALU ops (no ACT table functions)."""
            ti = rsqp.tile([p, w], dt.int32, tag=f"rsq_i{p}_{w}",
                           name=f"rsqi_{p}_{w}")
            nc.vector.tensor_scalar(
                out=ti, in0=src_ap.bitcast(dt.int32), scalar1=1, scalar2=None,
                op0=ALU.logical_shift_right)
            nc.vector.tensor_scalar(
                out=ti, in0=ti, scalar1=-1, scalar2=RSQC,
                op0=ALU.mult, op1=ALU.add)
            y = ti.bitcast(dt.float32)
            u = rsqp.tile([p, w], dt.float32, tag=f"rsq_u{p}_{w}",
                          name=f"rsqu_{p}_{w}")
            for _ in range(2):
                nc.vector.tensor_mul(out=u, in0=y, in1=y)
                nc.vector.scalar_tensor_tensor(
                    out=u, in0=u, scalar=-0.5, in1=src_ap,
                    op0=ALU.mult, op1=ALU.mult)
                nc.vector.scalar_tensor_tensor(
                    out=dst, in0=u, scalar=1.5, in1=y,
                    op0=ALU.add, op1=ALU.mult)
                y = dst
            return dst

        # ================= Phase B: packed keys -> kn, v =================
        bctx = ExitStack()
        with bctx:
            xbp = bctx.enter_context(tc.tile_pool(name="xbp", bufs=2))
            xknp = bctx.enter_context(tc.tile_pool(name="xknp", bufs=5))
            stp = bctx.enter_context(tc.tile_pool(name="stp", bufs=4))
            rbp = bctx.enter_context(tc.tile_pool(name="rbp", bufs=2))
            q2p = bctx.enter_context(tc.tile_pool(name="q2p", bufs=2))
            trB = bctx.enter_context(tc.tile_pool(name="trB", bufs=2, space="PSUM"))
            pskp = bctx.enter_context(tc.tile_pool(name="pskp", bufs=2, space="PSUM"))
            sskp = bctx.enter_context(tc.tile_pool(name="sskp", bufs=2, space="PSUM"))
            psvp = bctx.enter_context(tc.tile_pool(name="psvp", bufs=2, space="PSUM"))
            xkT_pool = bctx.enter_context(tc.tile_pool(name="xkT", bufs=1))
            xkT = [xkT_pool.tile([128, JN], dt.float32r, tag=f"xkT{ct}",
                                 name=f"xkT{ct}") for ct in range(CT)]

            # input loads: first group per-tile (fast availability), then
            # grouped; interleave const/weight DMAs by first-use time
            xg = []
            xt0 = xbp.tile([128, 4, DIM], dt.float32, tag="xt", name="xtB0")
            for k in range(4):
                nc.sync.dma_start(
                    out=xt0[:, k, :],
                    in_=xp_d[k * 128 : (k + 1) * 128, :])
                if k == 1:
                    nc.sync.dma_start(out=gk_sb, in_=gk_d[:, :])
                    nc.sync.dma_start(out=gk2i_sb, in_=gk2i_d[:, :])
            xg.append(xt0)
            nc.sync.dma_start(out=gq_sb, in_=gq_d[:, :])
            nc.sync.dma_start(out=gq2i_sb, in_=gq2i_d[:, :])
            nc.sync.dma_start(out=mbk_sb, in_=mbk_d[:, :])
            for gi, ntiles in ((1, 4), (2, 1)):
                xt = xbp.tile([128, 4, DIM], dt.float32, tag="xt",
                              name=f"xtB{gi}")
                nc.sync.dma_start(
                    out=xt[:, 0:ntiles, :],
                    in_=bass.AP(
                        tensor=xp_d,
                        offset=gi * 4 * 128 * DIM,
                        ap=[[DIM, 128], [128 * DIM, ntiles], [1, DIM]],
                    ),
                )
                xg.append(xt)
            wk_sb = wts_pool.tile([128, CT, E], dt.float32r, tag="wk")
            nc.gpsimd.dma_start(
                out=wk_sb,
                in_=wk_d.rearrange("(ct p) e -> p ct e", p=128).bitcast(dt.float32r))

            # LN: mean-subtract; collect var+eps for the v-path rstd
            varc = const.tile([128, JT], dt.float32, tag="varc")
            xkn = []
            for c in range(JT):
                xt = xg[c // 4][:, c % 4, :]
                st = stp.tile([128, 2, 6], dt.float32, tag="st")
                nc.vector.bn_stats(out=st[:, 0, :], in_=xt[:, 0:512])
                nc.vector.bn_stats(out=st[:, 1, :], in_=xt[:, 512:1024])
                mv = stp.tile([128, 3], dt.float32, tag="mv")
                nc.vector.bn_aggr(out=mv[:, 0:2], in_=st)
                nc.vector.tensor_scalar_add(
                    out=varc[:, c : c + 1], in0=mv[:, 1:2], scalar1=1e-5)
                nc.vector.tensor_scalar_mul(
                    out=mv[:, 2:3], in0=mv[:, 0:1], scalar1=-1.0)
                xnt = xknp.tile([128, DIM], dt.float32r, tag="xkn")
                nc.scalar.activation(xnt, xt, AF.Identity, bias=mv[:, 2:3])
                xkn.append(xnt)

            # interleaved per chunk: transposes -> k-proj -> sumsq -> rsqrt
            # chain (kn chunks become ready progressively)
            nco = 0
            for ci, c0, w in KCHUNKS:
                for ct in range(CT):
                    bank = trB.tile([128, 512], dt.float32r, tag="bank")
                    for k in range(w // 128):
                        nc.tensor.transpose(
                            out=bank[:, k * 128 : (k + 1) * 128],
                            in_=xkn[ci * 4 + k][:, ct * 128 : (ct + 1) * 128],
                            identity=identb,
                        )
                    if nco % 2 == 0:
                        nc.scalar.copy(
                            out=xkT[ct][:, c0 : c0 + w], in_=bank[:, 0:w])
                    else:
                        nc.vector.tensor_copy(
                            out=xkT[ct][:, c0 : c0 + w], in_=bank[:, 0:w])
                    nco += 1
                q2cs = []
                for et in range(2):
                    psk = pskp.tile([128, 512], dt.float32, tag="psk")
                    for ct in range(CT):
                        nc.tensor.matmul(
                            psk[:, 0:w],
                            wk_sb[:, ct, et * 128 : (et + 1) * 128],
                            xkT[ct][:, c0 : c0 + w],
                            start=(ct == 0), stop=(ct == CT - 1),
                        )
                    nc.vector.tensor_scalar_mul(
                        out=kn[et][:, c0 : c0 + w], in0=psk[:, 0:w],
                        scalar1=gk_sb[:, et : et + 1],
                    )
                    q2c = q2p.tile([128, 512], dt.float32r, tag="q2c")
                    nc.vector.scalar_tensor_tensor(
                        out=q2c[:, 0:w], in0=kn[et][:, c0 : c0 + w],
                        scalar=gk2i_sb[:, et : et + 1],
                        in1=kn[et][:, c0 : c0 + w],
                        op0=ALU.mult, op1=ALU.mult)
                    q2cs.append(q2c)
                ns = w // 128
                ssk_t = sskp.tile([128, 4, 4], dt.float32, tag="sskT",
                                  name=f"sskT{ci}")
                for s in range(ns):
                    for et in range(2):
                        nc.tensor.matmul(
                            ssk_t[:, s, :],
                            q2cs[et][:, s * 128 : (s + 1) * 128],
                            blk[et],
                            start=(s == 0 and et == 0),
                            stop=(s == ns - 1 and et == 1),
                            skip_group_check=True,
                        )
                ssk_s = rbp.tile([128, 4, 4], dt.float32r, tag="ssks")
                nc.vector.tensor_copy(
                    out=ssk_s[:, 0:ns, :], in_=ssk_t[:, 0:ns, :])
                trb2 = trB.tile([128, 512], dt.float32r, tag="bank",
                                name=f"trbsr{ci}")
                nc.tensor.transpose(
                    out=trb2[0 : ns * 4, 0:128],
                    in_=ssk_s.rearrange("p a b -> p (a b)")[:, 0 : ns * 4],
                    identity=identb)
                srk = srp.tile([16, 128], dt.float32, tag="srk")
                nc.vector.tensor_copy(
                    out=srk[0 : ns * 4, :],
                    in_=trb2[0 : ns * 4, 0:128].bitcast(dt.float32))
                rskT = srp.tile([16, 128], dt.float32, tag="rskT")
                emit_rsqrt(rskT[0 : ns * 4, :], srk[0 : ns * 4, :], 128,
                           p=ns * 4)
                nc.gpsimd.dma_start(
                    out=bass.AP(
                        tensor=rstdk_dram.tensor,
                        offset=rstdk_dram.offset + c0,
                        ap=[[128, ns], [JN, 4], [1, 128]],
                    ),
                    in_=rskT[0 : ns * 4, :])
                for et in range(2):
                    rb = rbp.tile([128, 512], dt.float32, tag="rb")
                    nc.gpsimd.dma_start(
                        out=rb[:, 0:w],
                        in_=bass.AP(
                            tensor=rstdk_dram.tensor,
                            offset=rstdk_dram.offset + 2 * et * JN + c0,
                            ap=[[JN, 2], [0, 64], [1, w]],
                        ),
                    )
                    sl = kn[et][:, c0 : c0 + w]
                    nc.vector.tensor_mul(out=sl, in0=sl, in1=rb[:, 0:w])

            vstd = const.tile([128, JT], dt.float32, tag="vstd")
            emit_rsqrt(vstd, varc, JT)
            wv_sb = wts_pool.tile([128, CT, E], dt.float32r, tag="wv")
            nc.gpsimd.dma_start(
                out=wv_sb,
                in_=wv_d.rearrange("(ct p) e -> p ct e", p=128).bitcast(dt.float32r))
            wq_sb = wts_pool.tile([128, CT, E], dt.float32r, tag="wq")
            nc.gpsimd.dma_start(
                out=wq_sb,
                in_=wq_d.rearrange("(ct p) e -> p ct e", p=128).bitcast(dt.float32r))
            # v projection
            for c in range(JT):
                psv = psvp.tile([128, E], dt.float32, tag="psv")
                for ct in range(CT):
                    nc.tensor.matmul(
                        psv,
                        xkT[ct][:, c * 128 : (c + 1) * 128],
                        wv_sb[:, ct, :],
                        start=(ct == 0), stop=(ct == CT - 1),
                    )
                nc.vector.tensor_scalar_mul(
                    out=v_sb[c][:, :, 0:64],
                    in0=psv.rearrange("p (h d) -> p h d", d=64),
                    scalar1=vstd[:, c : c + 1],
                )
                nc.vector.tensor_copy(out=v_sb[c][:, :, 64], in_=ones_bf)

            wo_sb = wts_pool.tile([128, 2, DIM], dt.bfloat16, tag="wo")
            nc.gpsimd.dma_start(out=wo_sb,
                              in_=wo_d.rearrange("(et p) c -> p et c", p=128))

        # ================= Phase C: query chunks =================
        xqp = ctx.enter_context(tc.tile_pool(name="xqp", bufs=2))
        xqnp = ctx.enter_context(tc.tile_pool(name="xqnp", bufs=8))
        stq = ctx.enter_context(tc.tile_pool(name="stq", bufs=4))
        rbq = ctx.enter_context(tc.tile_pool(name="rbq", bufs=2))
        q2q = ctx.enter_context(tc.tile_pool(name="q2q", bufs=2))
        expp = ctx.enter_context(tc.tile_pool(name="expp", bufs=4))
        dnp = ctx.enter_context(tc.tile_pool(name="dnp", bufs=3))
        rdp = ctx.enter_context(tc.tile_pool(name="rdp", bufs=2))
        otp = ctx.enter_context(tc.tile_pool(name="otp", bufs=3))
        trC = ctx.enter_context(tc.tile_pool(name="trC", bufs=1, space="PSUM"))
        ssqp = ctx.enter_context(tc.tile_pool(name="ssqp", bufs=1, space="PSUM"))
        scp = ctx.enter_context(tc.tile_pool(name="scp", bufs=2, space="PSUM"))
        avp = ctx.enter_context(tc.tile_pool(name="avp", bufs=1, space="PSUM"))

        def emit_qside(icn):
            """LN + transpose + q-projection + rms-norm for query chunk icn."""
            c0 = icn * 512
            xnTc = xntp.tile([128, CT, 512], dt.float32r, tag="xnTc",
                             name=f"xnTc{icn}")
            xt = xqp.tile([128, 4, DIM], dt.float32, tag="xt", name=f"xtC{icn}")
            nc.sync.dma_start(
                out=xt,
                in_=bass.AP(
                    tensor=x_d,
                    offset=c0 * DIM,
                    ap=[[DIM, 128], [128 * DIM, 4], [1, DIM]],
                ),
            )
            xqn = []
            for k in range(4):
                xnt = xqnp.tile([128, DIM], dt.float32r, tag="xqn")
                if icn < 2:
                    st = stq.tile([128, 2], dt.float32, tag="sta")
                    nc.scalar.activation(xnt, xt[:, k, :], AF.Copy,
                                         accum_out=st[:, 0:1])
                    nc.vector.tensor_scalar_mul(
                        out=st[:, 1:2], in0=st[:, 0:1], scalar1=-1.0 / DIM)
                    nc.vector.tensor_scalar_add(out=xnt, in0=xnt,
                                                scalar1=st[:, 1:2])
                else:
                    st = stq.tile([128, 2, 6], dt.float32, tag="st")
                    nc.vector.bn_stats(out=st[:, 0, :], in_=xt[:, k, 0:512])
                    nc.vector.bn_stats(out=st[:, 1, :], in_=xt[:, k, 512:1024])
                    mv = stq.tile([128, 3], dt.float32, tag="mv")
                    nc.vector.bn_aggr(out=mv[:, 0:2], in_=st)
                    nc.vector.tensor_scalar_mul(
                        out=mv[:, 2:3], in0=mv[:, 0:1], scalar1=-1.0)
                    nc.vector.tensor_scalar_add(out=xnt, in0=xt[:, k, :],
                                                scalar1=mv[:, 2:3])
                xqn.append(xnt)
            for ct in range(CT):
                bank = trC.tile([128, 512], dt.float32r, tag="bank")
                for k in range(4):
                    nc.tensor.transpose(
                        out=bank[:, k * 128 : (k + 1) * 128],
                        in_=xqn[k][:, ct * 128 : (ct + 1) * 128],
                        identity=identb,
                    )
                nc.vector.tensor_copy(out=xnTc[:, ct, :], in_=bank)

            qts = []
            ssq = ssqp.tile([128, 4, 4], dt.float32, tag="ssq")
            for et in range(2):
                psq = trC.tile([128, 512], dt.float32r, tag="bank",
                               name=f"psq{icn}_{et}").bitcast(dt.float32)
                for ct in range(CT):
                    nc.tensor.matmul(
                        psq,
                        wq_sb[:, ct, et * 128 : (et + 1) * 128],
                        xnTc[:, ct, :],
                        start=(ct == 0), stop=(ct == CT - 1),
                    )
                qt = qnp.tile([128, 512], dt.float32r, tag=f"qn{et}",
                              name=f"qn{et}_{icn}")
                nc.vector.tensor_scalar_mul(
                    out=qt, in0=psq, scalar1=gq_sb[:, et : et + 1])
                q2c = q2q.tile([128, 512], dt.float32r, tag="q2c")
                nc.vector.scalar_tensor_tensor(
                    out=q2c, in0=qt, scalar=gq2i_sb[:, et : et + 1], in1=qt,
                    op0=ALU.mult, op1=ALU.mult)
                for s in range(4):
                    nc.tensor.matmul(
                        ssq[:, s, :],
                        q2c[:, s * 128 : (s + 1) * 128],
                        blk[et],
                        start=(et == 0 and s == 0),
                        stop=(et == 1 and s == 3),
                        skip_group_check=True,
                    )
                qts.append(qt)
            ssq_s = rbq.tile([128, 4, 4], dt.float32r, tag="ssqs")
            nc.vector.tensor_copy(out=ssq_s, in_=ssq)
            trb2 = trC.tile([128, 512], dt.float32r, tag="bank",
                            name=f"trcsr{icn}")
            nc.tensor.transpose(
                out=trb2[0:16, 0:128],
                in_=ssq_s.rearrange("p a b -> p (a b)"),
                identity=identb)
            srq = srp.tile([16, 128], dt.float32, tag="srq")
            nc.vector.tensor_copy(out=srq,
                                  in_=trb2[0:16, 0:128].bitcast(dt.float32))
            rsqT = srp.tile([16, 128], dt.float32, tag="rsqT")
            emit_rsqrt(rsqT, srq, 128, p=16)
            nc.sync.dma_start(
                out=bass.AP(
                    tensor=rstdq_dram.tensor,
                    offset=rstdq_dram.offset + c0,
                    ap=[[128, 4], [N, 4], [1, 128]],
                ),
                in_=rsqT)
            for et in range(2):
                rb = rbq.tile([128, 512], dt.float32, tag="rb")
                nc.sync.dma_start(
                    out=rb,
                    in_=bass.AP(
                        tensor=rstdq_dram.tensor,
                        offset=rstdq_dram.offset + 2 * et * N + c0,
                        ap=[[N, 2], [0, 64], [1, 512]],
                    ),
                )
                nc.vector.tensor_mul(out=qts[et], in0=qts[et], in1=rb)
            return qts

        def emit_outproj(icn, outn):
            c0 = icn * 512
            for ct in range(CT):
                po = trC.tile([128, 512], dt.float32r, tag="bank",
                              name=f"po{icn}_{ct}").bitcast(dt.float32)
                for et in range(2):
                    nc.tensor.matmul(
                        po,
                        wo_sb[:, et, ct * 128 : (ct + 1) * 128],
                        outn[et],
                        start=(et == 0), stop=(et == 1),
                    )
                ott = otp.tile([128, 512], dt.bfloat16, tag="ot")
                nc.vector.tensor_copy(out=ott, in_=po)
                nc.sync.dma_start(
                    out=out_d[ct * 128 : (ct + 1) * 128, c0 : c0 + 512],
                    in_=ott,
                )

        qn_pipe = [emit_qside(0), emit_qside(1)]
        pending = None
        for icn in range(4):
            if icn < 2:
                qn_pipe.append(emit_qside(icn + 2))
            qn_cur = qn_pipe[icn]
            c0 = icn * 512
            outn = []
            for et in range(2):
                if et == 1 and pending is not None:
                    emit_outproj(*pending)
                    pending = None
                avA = avp.tile([65, 512], dt.float32, tag="avA")
                avB = avp.tile([65, 512], dt.float32, tag="avB")

                def emit_score(jt):
                    j0 = jt * 128
                    sc = scp.tile([128, 1024], dt.float32, tag="sc",
                                  name=f"sc{icn}_{et}_{jt}")
                    nc.tensor.matmul(
                        sc[:, 0:512], kn[et][0:64, j0 : j0 + 128],
                        qn_cur[et][0:64, :], start=True, stop=True)
                    nc.tensor.matmul(
                        sc[:, 512:1024], kn[et][64:128, j0 : j0 + 128],
                        qn_cur[et][64:128, :], start=True, stop=True)
                    return sc

                sc_next = emit_score(0)
                for jt in range(JT):
                    sc = sc_next
                    ex = expp.tile([128, 1024], dt.bfloat16, tag="ex")
                    nc.scalar.activation(
                        ex, sc, AF.Exp, bias=mbk_sb[:, jt : jt + 1])
                    if jt + 1 < JT:
                        sc_next = emit_score(jt + 1)
                    st_, sp_ = (jt == 0), (jt == JT - 1)
                    nc.tensor.matmul(
                        avA, v_sb[jt][:, 2 * et, :], ex[:, 0:512],
                        start=st_, stop=sp_, skip_group_check=True)
                    nc.tensor.matmul(
                        avB, v_sb[jt][:, 2 * et + 1, :], ex[:, 512:1024],
                        start=st_, stop=sp_, skip_group_check=True)
                # copy av banks to SBUF immediately (frees the PSUM banks so
                # the next head-pair's accumulation can start), then do the
                # denominator round-trip from the SBUF copy
                avs = dnp.tile([128, 1024], dt.float32, tag="dn")
                nc.vector.tensor_copy(out=avs[0:65, 0:512], in_=avA)
                nc.vector.tensor_copy(out=avs[0:65, 512:1024], in_=avB)
                ddr = drams.tile([2, 512], dt.float32, tag=f"ddr{icn}{et}",
                                 name=f"ddr{icn}{et}")
                nc.sync.dma_start(
                    out=ddr.rearrange("a b -> (a b)")[None, :],
                    in_=avs[64:65, :])
                rbden = rdp.tile([64, 2, 512], dt.float32, tag="rbden")
                for h in range(2):
                    nc.sync.dma_start(
                        out=rbden[:, h, :],
                        in_=bass.AP(
                            tensor=ddr.tensor,
                            offset=ddr.offset + h * 512,
                            ap=[[0, 64], [1, 512]],
                        ),
                    )
                nc.vector.reciprocal_approx_fast(out=rbden, in_=rbden)
                ot = onp.tile([128, 512], dt.bfloat16, tag=f"on{et}",
                              name=f"on{et}_{icn}")
                for h in range(2):
                    nc.vector.tensor_mul(
                        out=ot[h * 64 : (h + 1) * 64, :],
                        in0=avs[0:64, h * 512 : (h + 1) * 512],
                        in1=rbden[:, h, :])
                outn.append(ot)
            pending = (icn, outn)
        emit_outproj(*pending)


def _prep_inputs(x, mask, gamma_ln, gamma_q, gamma_k, Wq, Wkv, Wo):
    x = np.asarray(x, dtype=np.float32)
    mask = np.asarray(mask)
    gamma_ln = np.asarray(gamma_ln, dtype=np.float32)
    gamma_q = np.asarray(gamma_q, dtype=np.float32)
    gamma_k = np.asarray(gamma_k, dtype=np.float32)
    Wq = np.asarray(Wq, dtype=np.float32)
    Wkv = np.asarray(Wkv, dtype=np.float32)
    Wo = np.asarray(Wo, dtype=np.float32)

    Wqg = np.ascontiguousarray(Wq * gamma_ln[:, None])
    Wk = np.ascontiguousarray(Wkv[:, :DIM] * gamma_ln[:, None])
    Wv = np.ascontiguousarray(Wkv[:, DIM:] * gamma_ln[:, None])

    gq_full = (np.float32(np.sqrt(D)) * gamma_q.reshape(HEADS, D)).astype(np.float32)
    gk_full = (np.float32(np.sqrt(D)) * gamma_k.reshape(HEADS, D)).astype(np.float32)

    packs = []
    for b in range(B):
        valid = np.flatnonzero(mask[b])
        nv = len(valid)
        assert nv <= JN, f"valid keys {nv} > JN={JN}; raise JT"
        idx = np.zeros(JN, np.int64)
        idx[:nv] = valid
        xp = np.ascontiguousarray(x[b][idx])
        mb = np.where(np.arange(JN) < nv, np.float32(0.0),
                      np.float32(NEG)).astype(np.float32)
        mbk = np.ascontiguousarray(mb.reshape(JT, 128).T)
        packs.append((xp, mbk))

    in_maps = []
    for core in range(8):
        b, g = divmod(core, G)
        sl = slice(g * E, (g + 1) * E)
        gq = gq_full[g * HPG : (g + 1) * HPG].reshape(E)
        gk = gk_full[g * HPG : (g + 1) * HPG].reshape(E)
        gq2 = gq.reshape(2, 128).T
        gk2 = gk.reshape(2, 128).T
        xp, mbk = packs[b]
        in_maps.append({
            "x": np.ascontiguousarray(x[b]),
            "xp": xp,
            "mbk": mbk,
            "wq": np.ascontiguousarray(Wqg[:, sl]),
            "wk": np.ascontiguousarray(Wk[:, sl]),
            "wv": np.ascontiguousarray(Wv[:, sl]),
            "wo": np.ascontiguousarray(Wo[sl, :]).astype(ml_dtypes.bfloat16),
            "gq": np.ascontiguousarray(gq2),
            "gk": np.ascontiguousarray(gk2),
            "gq2i": np.ascontiguousarray(1.0 / (gq2 * gq2)),
            "gk2i": np.ascontiguousarray(1.0 / (gk2 * gk2)),
        })
    return in_maps


def kernel(x, mask, gamma_ln, gamma_q, gamma_k, Wq, Wkv, Wo, _trace=False):
    if "nc" not in _CACHE:
        _CACHE["nc"] = _build_nc()
    nc = _CACHE["nc"]
    in_maps = _prep_inputs(x, mask, gamma_ln, gamma_q, gamma_k, Wq, Wkv, Wo)
    try:
        res = run_bass_kernel_spmd(nc, in_maps, core_ids=list(range(8)),
                                   trace=_trace)
    except Exception:
        # axon terminals occasionally surface transient device errors from
        # earlier sessions; one retry on a fresh attempt is reliable
        res = run_bass_kernel_spmd(nc, in_maps, core_ids=list(range(8)),
                                   trace=_trace)
    _CACHE["last_result"] = res
    out = np.zeros((B, N, DIM), dtype=np.float32)
    for core in range(8):
        b = core // G
        out[b] += res.results[core]["outT"].astype(np.float32).T
    return out
